# revision 1
# baseline (speedup 1.0000x reference)
"""Local sliding-window attention block (MQA + partial RoPE) on 8 TRN2 cores.

Sharding: 2 batches x 4 sequence chunks of 512 queries each. Each core
computes q/k/v projections for its chunk (keys include a 512-token halo),
windowed attention (window=512, causal), and the o-projection for its own
query rows — so the host-side unshard is a pure concatenation.

On-chip layout: everything transposed (feature dim on partitions).
  xT[d, pos]  ->  Q^T[dh, q] / K^T[dh, k] (RoPE'd)  ->  S^T[k, q]
  -> exp -> P^T[k, q] (bf16, multiplicative 0/1 masks)
  -> O^T[dv, q] = V.T-matmul  -> normalized by softmax denominators
     (partition_all_reduce on GPSIMD)  -> used directly as lhsT of o-proj.
All matmuls bf16 inputs, fp32 PSUM accumulation.

Schedule notes (tuned against the CoreSim cost model):
 - weight/const DMAs are host-pre-laid-out to match SBUF (>=512B rows,
   full DMA bus rate) and ordered so the K/V projection starts on the
   first xT tile; per-head Wq streams 2 heads ahead with 3 buffers.
 - V is projected transposed (one PSUM accumulation group per bank) and
   flipped to [pos, dv] tiles with DMA xbar transposes.
 - per head the PE stream is software-pipelined: scores(t0,t1) -> next
   head's q-projection -> scores(t2,t3) -> all AV matmuls, so the
   exp/mask chain on Act/DVE hides under the q-projection. The last
   head interleaves heads 0..14 of the first two o-proj units instead.
 - softmax sig-reduction is an fp16 add-tree (2-byte packed DVE ops run
   2x, first add on GPSIMD) instead of a strided TensorReduce.
 - rope half-swaps ride the GPSIMD SWDGE queue so the SP load queue is
   never blocked behind data-dependent transfers.
 - the whole o-projection runs out of the score-PSUM banks (no pool
   transition barrier); the final unit is split so the trailing
   bias-add + store latency shrinks.
"""

import numpy as np
import ml_dtypes

BF16 = ml_dtypes.bfloat16

B, L, D = 2, 2048, 2048
H, HD = 16, 128
ROPE_DIMS, HALF = 64, 32
WINDOW = 512
ROPE_BASE = 10000.0
SCALE = HD ** -0.5

CHUNK = 512            # queries per core
NK = 1024              # keys (incl. halo) per core
NQT = CHUNK // 128     # 4 local query tiles
NKT = NK // 128        # 8 local key tiles
NSIG = 5               # key tiles in window per query tile
NDT = D // 128         # 16 contraction tiles over embedding dim
DN = D // 512          # 4 o-proj column blocks

_PROGRAM = None


def _build_program():
    from contextlib import ExitStack
    import concourse.bass as bass
    import concourse.mybir as mybir
    import concourse.tile as tile
    import concourse.bass_isa as bass_isa
    from concourse import bacc

    fp32 = mybir.dt.float32
    fp16 = mybir.dt.float16
    bf16 = mybir.dt.bfloat16
    AF = mybir.ActivationFunctionType

    nc = bacc.Bacc(None, target_bir_lowering=False)

    xT_d = nc.dram_tensor("xT", [D, NK], bf16, kind="ExternalInput")
    wq_d = nc.dram_tensor("Wq", [H, 128, NDT, 128], bf16, kind="ExternalInput")
    wk_d = nc.dram_tensor("Wk", [128, NDT, HD], bf16, kind="ExternalInput")
    wv_d = nc.dram_tensor("Wv", [128, NDT, HD], bf16, kind="ExternalInput")
    wo_d = nc.dram_tensor("Wo", [DN, 128, H, 512], bf16, kind="ExternalInput")
    bo_d = nc.dram_tensor("bo", [1, D], bf16, kind="ExternalInput")
    cos_d = nc.dram_tensor("cosT", [ROPE_DIMS, NK], bf16, kind="ExternalInput")
    sin_d = nc.dram_tensor("sinT", [ROPE_DIMS, NK], bf16, kind="ExternalInput")
    msk_d = nc.dram_tensor("masks", [128, NQT, NSIG, 128], bf16, kind="ExternalInput")
    out_d = nc.dram_tensor("out", [CHUNK, D], bf16, kind="ExternalOutput")

    def _rope(pool, out_bf, ps, cos2, sin2m):
        """out[0:64] = rotary(ps[0:64]); out[64:128] = ps[64:128].

        ps fp32 PSUM, out bf16. cos2/sin2m bf16 [64, n] row tables
        (rows [0:32]==[32:64]==cos; sin rows [0:32]=-sin, [32:64]=+sin).
        The half-swap goes through two partition-shifting DMAs; the
        PSUM->SBUF casts run on Act, the bf16 elementwise math runs on
        DVE in the 2x packed-16-bit mode.
        """
        n = cos2.shape[-1]
        sb64 = pool.tile([ROPE_DIMS, n], bf16, tag="rope_sb64")
        nc.scalar.copy(sb64, ps[0:ROPE_DIMS])
        ss = pool.tile([ROPE_DIMS, n], bf16, tag="rope_ss")
        # SWDGE (gpsimd) queue: keeps these data-dependent shuffles out of
        # the SP load queue so weight streaming is never blocked behind them
        nc.gpsimd.dma_start(out=ss[0:HALF], in_=sb64[HALF:ROPE_DIMS])
        nc.gpsimd.dma_start(out=ss[HALF:ROPE_DIMS], in_=sb64[0:HALF])
        t1 = pool.tile([ROPE_DIMS, n], bf16, tag="rope_t1")
        nc.vector.tensor_mul(t1, sb64, cos2)
        nc.vector.tensor_mul(ss, ss, sin2m)
        nc.vector.tensor_add(out_bf[0:ROPE_DIMS], t1, ss)
        nc.scalar.copy(out_bf[ROPE_DIMS:HD], ps[ROPE_DIMS:HD])

    with tile.TileContext(nc) as tc, ExitStack() as ctx:
        p_const = ctx.enter_context(tc.tile_pool(name="const", bufs=1))
        p_xt = ctx.enter_context(tc.tile_pool(name="xt", bufs=1))
        p_kv = ctx.enter_context(tc.tile_pool(name="kv", bufs=1))
        p_wq = ctx.enter_context(tc.tile_pool(name="wq", bufs=3))
        p_qt = ctx.enter_context(tc.tile_pool(name="qt", bufs=3))
        p_es = ctx.enter_context(tc.tile_pool(name="es", bufs=6))
        p_red = ctx.enter_context(tc.tile_pool(name="red", bufs=8))
        p_dn = ctx.enter_context(tc.tile_pool(name="dn", bufs=2))
        p_tmp = ctx.enter_context(tc.tile_pool(name="tmp", bufs=4))
        p_otn = ctx.enter_context(tc.tile_pool(name="otn", bufs=1))
        p_wo = ctx.enter_context(tc.tile_pool(name="wo", bufs=2))
        p_ob = ctx.enter_context(tc.tile_pool(name="ob", bufs=4))

        # ---- loads, ordered so compute starts ASAP ----
        wk_sb = p_const.tile([128, NDT, HD], bf16, tag="wk")
        nc.sync.dma_start(out=wk_sb, in_=wk_d[:])

        xview = xT_d[:].rearrange("(n p) m -> n p m", p=128)
        xt = []
        for i in range(NDT):
            t_ = p_xt.tile([128, NK], bf16, tag=f"xt{i}")
            xt.append(t_)
        nc.sync.dma_start(out=xt[0], in_=xview[0])

        wv_sb = p_const.tile([128, NDT, HD], bf16, tag="wv")
        nc.sync.dma_start(out=wv_sb, in_=wv_d[:])
        for i in range(1, NDT):
            nc.sync.dma_start(out=xt[i], in_=xview[i])

        wq_sb = []
        for h in range(H):
            t_ = p_wq.tile([128, NDT, 128], bf16, tag="wq", name=f"wq{h}")
            wq_sb.append(t_)
        nc.sync.dma_start(out=wq_sb[0], in_=wq_d[0])
        nc.sync.dma_start(out=wq_sb[1], in_=wq_d[1])

        cos_sb = p_const.tile([ROPE_DIMS, NK], bf16, tag="cos")
        nc.sync.dma_start(out=cos_sb, in_=cos_d[:])
        sin_sb = p_const.tile([ROPE_DIMS, NK], bf16, tag="sin")
        nc.sync.dma_start(out=sin_sb, in_=sin_d[:])

        msk_sb = p_const.tile([128, NQT, NSIG, 128], bf16, tag="msk")
        nc.sync.dma_start(out=msk_sb, in_=msk_d[:])

        # ---- K^T (RoPE'd) and V projections, dt-outer so PE starts on the
        # first xT tile while the rest stream in; head-0 q-projection is
        # folded in before the PSUM pool swap so PE never drains ----
        kt = p_kv.tile([128, NK], bf16, tag="kt")
        vt_sb = p_kv.tile([128, NK], bf16, tag="vt")
        v_sb = []
        for s in range(NKT):
            t_ = p_kv.tile([128, HD], bf16, tag=f"v{s}")
            v_sb.append(t_)
        qt_sb = []
        for h in range(H):
            t_ = p_qt.tile([128, CHUNK], bf16, tag="qt", name=f"qt{h}")
            qt_sb.append(t_)

        def q_proj(ps_qp, h):
            psq = ps_qp.tile([128, CHUNK], fp32, tag="ps_q")
            for dt in range(NDT):
                nc.tensor.matmul(
                    psq, wq_sb[h][:, dt, :], xt[dt][:, CHUNK:NK],
                    start=(dt == 0), stop=(dt == NDT - 1),
                )
            _rope(p_tmp, qt_sb[h], psq, cos_sb[:, CHUNK:NK], sin_sb[:, CHUNK:NK])

        with tc.tile_pool(name="ps_q", bufs=2, space=bass.MemorySpace.PSUM) as ps_qp:
            with tc.tile_pool(
                name="ps_kv", bufs=1, space=bass.MemorySpace.PSUM
            ) as ps_kv:
                ps_k = [
                    ps_kv.tile([128, 512], fp32, tag=f"ps_k{i}", name=f"ps_k{i}")
                    for i in range(2)
                ]
                # V is computed transposed (one PSUM accumulation group per
                # bank — concurrent groups within a bank are illegal) and
                # tile-transposed to [pos, dv] via the DMA xbar afterwards
                ps_vt = [
                    ps_kv.tile([128, 512], fp32, tag=f"ps_vt{i}", name=f"ps_vt{i}")
                    for i in range(2)
                ]
                for dt in range(NDT):
                    st, sp = dt == 0, dt == NDT - 1
                    for nh in range(2):
                        nc.tensor.matmul(
                            ps_k[nh], wk_sb[:, dt, :],
                            xt[dt][:, nh * 512:(nh + 1) * 512], start=st, stop=sp,
                        )
                        nc.tensor.matmul(
                            ps_vt[nh], wv_sb[:, dt, :],
                            xt[dt][:, nh * 512:(nh + 1) * 512], start=st, stop=sp,
                        )
                # head-0 q-projection keeps PE busy while K-rope / V copies
                # drain the kv PSUM tiles
                q_proj(ps_qp, 0)
                for nh in range(2):
                    cols = slice(nh * 512, (nh + 1) * 512)
                    _rope(p_tmp, kt[:, cols], ps_k[nh],
                          cos_sb[:, cols], sin_sb[:, cols])
                    nc.vector.tensor_copy(vt_sb[:, cols], ps_vt[nh])
                for c in range(NKT):
                    nc.sync.dma_start_transpose(
                        out=v_sb[c], in_=vt_sb[:, c * 128:(c + 1) * 128]
                    )

            # ---- per-head attention (software-pipelined PE stream) ----
            otn = []
            for h in range(H):
                t_ = p_otn.tile([128, CHUNK], bf16, tag=f"otn{h}")
                otn.append(t_)

            with (
                tc.tile_pool(name="ps_s", bufs=2, space=bass.MemorySpace.PSUM) as ps_sp,
                tc.tile_pool(name="ps_o", bufs=2, space=bass.MemorySpace.PSUM) as ps_op,
            ):
                bias_sb = p_const.tile([128, D], bf16, tag="bias")
                wo_tiles = []
                for n in range(DN):
                    t_ = p_wo.tile([128, H, 512], bf16, tag="wo", name=f"wo{n}")
                    wo_tiles.append(t_)

                for h in range(H):
                    if h == 12:
                        # prefetch o-proj operands so the tail phase starts hot
                        nc.sync.dma_start(
                            out=bias_sb,
                            in_=bass.AP(tensor=bo_d, offset=0, ap=[[0, 128], [1, D]]),
                        )
                    if h == 14:
                        nc.sync.dma_start(out=wo_tiles[0], in_=wo_d[0])
                    qt = qt_sb[h]
                    otp = ps_op.tile([128, CHUNK], fp32, tag="ps_o")
                    dn = p_dn.tile([128, NQT, 128], fp32, tag="dn")
                    ess = []

                    def attn_unit(t):
                        pss = ps_sp.tile([128, NSIG, 128], fp32, tag="ps_s")
                        qsl = qt[:, t * 128:(t + 1) * 128]
                        for sig in range(NSIG):
                            s = t + sig
                            nc.tensor.matmul(
                                pss[:, sig, :], kt[:, s * 128:(s + 1) * 128], qsl,
                                start=True, stop=True,
                            )
                        es = p_es.tile([128, NSIG, 128], bf16, tag="es")
                        ess.append(es)
                        nc.scalar.activation(es, pss, AF.Exp, scale=SCALE)
                        nc.vector.tensor_mul(es, es, msk_sb[:, t, :, :])
                        # fp16 add-tree: 2-byte packed DVE ops run 2x; exact
                        # zeros from the mask keep the boundary cores exact
                        r2 = p_red.tile([128, 2, 128], fp16, tag="r2")
                        nc.gpsimd.tensor_add(r2, es[:, 0:2, :], es[:, 2:4, :])
                        r1 = p_red.tile([128, 128], fp16, tag="r1")
                        nc.vector.tensor_add(r1, r2[:, 0, :], r2[:, 1, :])
                        red = p_red.tile([128, 128], fp16, tag="red")
                        nc.vector.tensor_add(red, r1, es[:, 4, :])
                        nc.gpsimd.partition_all_reduce(
                            dn[:, t, :], red, channels=128,
                            reduce_op=bass_isa.ReduceOp.add,
                        )

                    last_head = h == H - 1
                    pso_br = []
                    if last_head:
                        # the next-head q-projection no longer exists to cover
                        # the exp/mask chain; instead accumulate heads 0..14 of
                        # the first two o-proj units into the (free) ps_q
                        # slots, leaving the groups open until otn[15] lands
                        for i in range(2):
                            t_ = ps_qp.tile(
                                [128, CHUNK], fp32, tag="ps_q", name=f"ps_br{i}"
                            )
                            pso_br.append(t_)

                    def oproj_partial(i, hs):
                        for h2 in hs:
                            nc.tensor.matmul(
                                pso_br[i][:, 0:512],
                                otn[h2][:, i * 128:(i + 1) * 128],
                                wo_tiles[0][:, h2, :],
                                start=(h2 == 0), stop=(h2 == H - 1),
                            )

                    if h + 2 < H:
                        nc.sync.dma_start(out=wq_sb[h + 2], in_=wq_d[h + 2])
                    for t in range(2):
                        attn_unit(t)
                    if h + 1 < H:
                        q_proj(ps_qp, h + 1)
                    else:
                        oproj_partial(0, range(H - 1))
                    for t in range(2, NQT):
                        attn_unit(t)
                    if last_head:
                        oproj_partial(1, range(H - 1))
                    for t in range(NQT):
                        for sig in range(NSIG):
                            nc.tensor.matmul(
                                otp[:, t * 128:(t + 1) * 128],
                                v_sb[t + sig], ess[t][:, sig, :],
                                start=(sig == 0), stop=(sig == NSIG - 1),
                            )
                    rview = dn.rearrange("p t q -> p (t q)")
                    nc.vector.reciprocal(rview, rview)
                    nc.vector.tensor_mul(otn[h], otp, rview)
                    if last_head:
                        for i in range(2):
                            oproj_partial(i, [H - 1])
                            ob = p_ob.tile([128, 512], bf16, tag="ob")
                            nc.vector.tensor_add(
                                ob, pso_br[i], bias_sb[:, 0:512]
                            )
                            nc.scalar.dma_start(
                                out=out_d[i * 128:(i + 1) * 128, 0:512], in_=ob
                            )

                def oproj_slice(pso, n, t, hs):
                    for h in range(H):
                        nc.tensor.matmul(
                            pso[:, hs],
                            otn[h][:, t * 128:(t + 1) * 128],
                            wo_tiles[n][:, h, hs],
                            start=(h == 0), stop=(h == H - 1),
                        )
                    ob = p_ob.tile([128, 512], bf16, tag="ob")
                    nc.vector.tensor_add(
                        ob[:, hs], pso[:, hs],
                        bias_sb[:, n * 512 + hs.start:n * 512 + hs.stop],
                    )
                    nc.scalar.dma_start(
                        out=out_d[
                            t * 128:(t + 1) * 128,
                            n * 512 + hs.start:n * 512 + hs.stop,
                        ],
                        in_=ob[:, hs],
                    )

                def oproj_unit(pso, n, t):
                    oproj_slice(pso, n, t, slice(0, 512))

                # ---- o-projection + bias, straight out of the (now idle)
                # score-PSUM banks — no pool transition barrier ----
                nc.sync.dma_start(out=wo_tiles[1], in_=wo_d[1])
                for t in range(2, NQT):
                    pst = ps_sp.tile([128, NSIG, 128], fp32, tag="ps_s")
                    pso = pst.rearrange("p s q -> p (s q)")[:, 0:512]
                    oproj_unit(pso, 0, t)
                for n in range(1, DN):
                    if n + 1 < DN:
                        nc.sync.dma_start(out=wo_tiles[n + 1], in_=wo_d[n + 1])
                    for t in range(NQT):
                        pst = ps_sp.tile([128, NSIG, 128], fp32, tag="ps_s")
                        pso = pst.rearrange("p s q -> p (s q)")[:, 0:512]
                        if n == DN - 1 and t == NQT - 1:
                            # final unit: the last slice goes in the OTHER
                            # ps_s slot so its matmuls don't wait for the
                            # first slice's bias-add read (same-tile hazard),
                            # and the trailing store chain is short
                            oproj_slice(pso, n, t, slice(0, 384))
                            pst2 = ps_sp.tile(
                                [128, NSIG, 128], fp32, tag="ps_s", name="pst2"
                            )
                            pso2 = pst2.rearrange("p s q -> p (s q)")[:, 0:512]
                            oproj_slice(pso2, n, t, slice(384, 512))
                        else:
                            oproj_unit(pso, n, t)

    nc.compile()
    return nc


def _get_program():
    global _PROGRAM
    if _PROGRAM is None:
        _PROGRAM = _build_program()
    return _PROGRAM


def _make_in_maps(x, Wq, Wk, Wv, Wo, bo):
    # host pre-layouts that mirror the SBUF tiles exactly (partition-major,
    # >=512B contiguous per partition) so every DMA runs at full bus rate
    Wq_b = np.ascontiguousarray(
        np.asarray(Wq, np.float32).reshape(NDT, 128, H, 128).transpose(2, 1, 0, 3)
    ).astype(BF16)
    Wk_b = np.ascontiguousarray(
        np.asarray(Wk, np.float32).reshape(NDT, 128, HD).transpose(1, 0, 2)
    ).astype(BF16)
    Wv_b = np.ascontiguousarray(
        np.asarray(Wv, np.float32).reshape(NDT, 128, HD).transpose(1, 0, 2)
    ).astype(BF16)
    Wo_b = np.ascontiguousarray(
        np.asarray(Wo, np.float32).reshape(H, 128, DN, 512).transpose(2, 1, 0, 3)
    ).astype(BF16)
    bo_f = np.ascontiguousarray(np.asarray(bo, np.float32).reshape(1, D)).astype(BF16)

    inv_freq = np.exp(
        -np.log(np.float32(ROPE_BASE))
        * (np.arange(0, ROPE_DIMS, 2, dtype=np.float32) / np.float32(ROPE_DIMS))
    ).astype(np.float32)

    in_maps = []
    for c in range(8):
        b, g = divmod(c, 4)
        k_start = 512 * g - 512
        xs = np.zeros((NK, D), np.float32)
        lo = max(0, k_start)
        xs[lo - k_start:] = x[b, lo:k_start + NK]
        xT = np.ascontiguousarray(xs.T).astype(BF16)

        pos = (k_start + np.arange(NK)).astype(np.float32)
        theta = pos[None, :] * inv_freq[:, None]          # [32, NK]
        cos2 = np.ascontiguousarray(
            np.concatenate([np.cos(theta)] * 2, axis=0)).astype(BF16)
        sin2 = np.ascontiguousarray(
            np.concatenate([-np.sin(theta), np.sin(theta)], axis=0)).astype(BF16)

        m = np.zeros((NQT, NSIG, 128, 128), np.float32)
        for t in range(NQT):
            Tg = NQT * g + t
            for sig in range(NSIG):
                S = Tg - 4 + sig
                if S < 0:
                    continue
                i = (128 * Tg + np.arange(128))[None, :]   # queries (cols)
                j = (128 * S + np.arange(128))[:, None]    # keys (rows)
                m[t, sig] = (((i - j) >= 0) & ((i - j) < WINDOW)).astype(np.float32)
        # SBUF layout [k, t, sig, q]
        masks = np.ascontiguousarray(m.transpose(2, 0, 1, 3)).astype(BF16)

        in_maps.append({
            "xT": xT, "Wq": Wq_b, "Wk": Wk_b, "Wv": Wv_b, "Wo": Wo_b,
            "bo": bo_f, "cosT": cos2, "sinT": sin2, "masks": masks,
        })
    return in_maps


def _unshard(results):
    out = np.zeros((B, L, D), np.float32)
    for c in range(8):
        b, g = divmod(c, 4)
        out[b, CHUNK * g:CHUNK * (g + 1)] = results[c]["out"]
    return out


def kernel(x, Wq, Wk, Wv, Wo, bo):
    from concourse.bass_utils import run_bass_kernel_spmd

    nc = _get_program()
    in_maps = _make_in_maps(x, Wq, Wk, Wv, Wo, bo)
    res = run_bass_kernel_spmd(nc, in_maps, core_ids=list(range(8)))
    return _unshard(res.results)



# revision 11
# speedup vs baseline: 1.0196x; 1.0196x over previous
"""Local sliding-window attention block (MQA + partial RoPE) on 8 TRN2 cores.

Sharding: 2 batches x 4 sequence chunks of 512 queries each. Each core
computes q/k/v projections for its chunk (keys include a 512-token halo),
windowed attention (window=512, causal), and the o-projection for its own
query rows — so the host-side unshard is a pure concatenation.

On-chip layout: everything transposed (feature dim on partitions).
  xT[d, pos]  ->  Q^T[dh, q] / K^T[dh, k] (RoPE'd)  ->  S^T[k, q]
  -> exp -> P^T[k, q] (bf16, multiplicative 0/1 masks)
  -> O^T[dv, q] = V.T-matmul  -> normalized by softmax denominators
     (partition_all_reduce on GPSIMD)  -> used directly as lhsT of o-proj.
All matmuls bf16 inputs, fp32 PSUM accumulation.

Schedule notes (tuned against the CoreSim cost model):
 - weight/const DMAs are host-pre-laid-out to match SBUF (>=512B rows,
   full DMA bus rate) and ordered so the K/V projection starts on the
   first xT tile; per-head Wq streams 2 heads ahead with 3 buffers.
 - V is projected transposed (one PSUM accumulation group per bank) and
   flipped to [pos, dv] tiles with DMA xbar transposes.
 - per head the PE stream is software-pipelined: scores(t0,t1) -> next
   head's q-projection -> scores(t2,t3) -> all AV matmuls, so the
   exp/mask chain on Act/DVE hides under the q-projection. The last
   head interleaves heads 0..14 of the first two o-proj units instead.
 - softmax sig-reduction is an fp16 add-tree (2-byte packed DVE ops run
   2x, first add on GPSIMD) instead of a strided TensorReduce.
 - rope half-swaps ride the GPSIMD SWDGE queue so the SP load queue is
   never blocked behind data-dependent transfers.
 - the whole o-projection runs out of the score-PSUM banks (no pool
   transition barrier); the final unit is split so the trailing
   bias-add + store latency shrinks.
"""

import numpy as np
import ml_dtypes

BF16 = ml_dtypes.bfloat16
F8 = ml_dtypes.float8_e4m3

B, L, D = 2, 2048, 2048
H, HD = 16, 128
ROPE_DIMS, HALF = 64, 32
WINDOW = 512
ROPE_BASE = 10000.0
SCALE = HD ** -0.5
# fp8 quantization scales: x is quantized at 16x, weights at 2048x, so every
# projection PSUM carries a 2^15 factor that the exp-scale / host descale absorb
SX = 16.0
SW = 2048.0
S2 = SX * SW  # 2^15

CHUNK = 512            # queries per core
NK = 1024              # keys (incl. halo) per core
NQT = CHUNK // 128     # 4 local query tiles
NKT = NK // 128        # 8 local key tiles
NSIG = 5               # key tiles in window per query tile
NDT = D // 128         # 16 contraction tiles over embedding dim
NPAIR = NDT // 2       # 8 dt pairs (256-deep DoubleRow contraction units)
DN = D // 512          # 4 o-proj column blocks

_PROGRAM = None


def _build_program():
    from contextlib import ExitStack
    import concourse.bass as bass
    import concourse.mybir as mybir
    import concourse.tile as tile
    import concourse.bass_isa as bass_isa
    from concourse import bacc

    fp32 = mybir.dt.float32
    fp16 = mybir.dt.float16
    bf16 = mybir.dt.bfloat16
    f8e4 = mybir.dt.float8e4
    DR = mybir.MatmulPerfMode.DoubleRow
    AF = mybir.ActivationFunctionType

    nc = bacc.Bacc(None, target_bir_lowering=False)

    # fp8 operands carry (hi, lo) compensation pairs: x tiles are laid out
    # [p, dt, (hi, lo), pos], weight tiles [p, dt, (lo, hi), col] so that a
    # single DoubleRow matmul over the hl axis yields the Wl.T@Xh + Wh.T@Xl
    # cross terms, and a DoubleRow over a dt pair at hl=hi yields the main
    # term with a 256-deep contraction at half the per-row cost.
    xT_d = nc.dram_tensor("xT8", [128, NDT, 2, NK], f8e4, kind="ExternalInput")
    wq_d = nc.dram_tensor("Wq", [H, 128, NDT, 2, 128], f8e4, kind="ExternalInput")
    wk_d = nc.dram_tensor("Wk", [128, NDT, 2, HD], f8e4, kind="ExternalInput")
    wv_d = nc.dram_tensor("Wv", [128, NDT, 2, HD], f8e4, kind="ExternalInput")
    wo_d = nc.dram_tensor("Wo", [DN, 128, H, 512], bf16, kind="ExternalInput")
    bo_d = nc.dram_tensor("bo", [1, D], bf16, kind="ExternalInput")
    cos_d = nc.dram_tensor("cosT", [ROPE_DIMS, NK], bf16, kind="ExternalInput")
    sin_d = nc.dram_tensor("sinT", [ROPE_DIMS, NK], bf16, kind="ExternalInput")
    msk_d = nc.dram_tensor("masks", [128, NQT, NSIG, 128], bf16, kind="ExternalInput")
    out_d = nc.dram_tensor("out", [CHUNK, D], bf16, kind="ExternalOutput")

    def _rope(pool, out_bf, ps, cos2, sin2m):
        """out[0:64] = rotary(ps[0:64]); out[64:128] = ps[64:128].

        ps fp32 PSUM, out bf16. cos2/sin2m bf16 [64, n] row tables
        (rows [0:32]==[32:64]==cos; sin rows [0:32]=-sin, [32:64]=+sin).
        The half-swap goes through two partition-shifting DMAs; the
        PSUM->SBUF casts run on Act, the bf16 elementwise math runs on
        DVE in the 2x packed-16-bit mode.
        """
        n = cos2.shape[-1]
        sb64 = pool.tile([ROPE_DIMS, n], bf16, tag="rope_sb64")
        nc.scalar.copy(sb64, ps[0:ROPE_DIMS])
        ss = pool.tile([ROPE_DIMS, n], bf16, tag="rope_ss")
        # SWDGE (gpsimd) queue: keeps these data-dependent shuffles out of
        # the SP load queue so weight streaming is never blocked behind them
        nc.gpsimd.dma_start(out=ss[0:HALF], in_=sb64[HALF:ROPE_DIMS])
        nc.gpsimd.dma_start(out=ss[HALF:ROPE_DIMS], in_=sb64[0:HALF])
        t1 = pool.tile([ROPE_DIMS, n], bf16, tag="rope_t1")
        nc.vector.tensor_mul(t1, sb64, cos2)
        nc.vector.tensor_mul(ss, ss, sin2m)
        nc.vector.tensor_add(out_bf[0:ROPE_DIMS], t1, ss)
        nc.scalar.copy(out_bf[ROPE_DIMS:HD], ps[ROPE_DIMS:HD])

    with tile.TileContext(nc) as tc, ExitStack() as ctx:
        p_const = ctx.enter_context(tc.tile_pool(name="const", bufs=1))
        p_xt = ctx.enter_context(tc.tile_pool(name="xt", bufs=1))
        p_kv = ctx.enter_context(tc.tile_pool(name="kv", bufs=1))
        p_wq = ctx.enter_context(tc.tile_pool(name="wq", bufs=3))
        p_qt = ctx.enter_context(tc.tile_pool(name="qt", bufs=3))
        p_es = ctx.enter_context(tc.tile_pool(name="es", bufs=6))
        p_red = ctx.enter_context(tc.tile_pool(name="red", bufs=8))
        p_dn = ctx.enter_context(tc.tile_pool(name="dn", bufs=2))
        p_tmp = ctx.enter_context(tc.tile_pool(name="tmp", bufs=4))
        p_otn = ctx.enter_context(tc.tile_pool(name="otn", bufs=1))
        p_wo = ctx.enter_context(tc.tile_pool(name="wo", bufs=2))
        p_ob = ctx.enter_context(tc.tile_pool(name="ob", bufs=4))

        # ---- loads, ordered so compute starts ASAP ----
        wk_sb = p_const.tile([128, NDT, 2, HD], f8e4, tag="wk")
        nc.sync.dma_start(out=wk_sb, in_=wk_d[:])

        # x in 8 dt-pair tiles [p, 2(dt), 2(hl), pos] so the first DoubleRow
        # starts after the first pair lands
        xp = []
        for i in range(NPAIR):
            t_ = p_xt.tile([128, 2, 2, NK], f8e4, tag=f"xp{i}")
            xp.append(t_)
        nc.sync.dma_start(out=xp[0], in_=xT_d[:, 0:2])

        wv_sb = p_const.tile([128, NDT, 2, HD], f8e4, tag="wv")
        nc.sync.dma_start(out=wv_sb, in_=wv_d[:])
        for i in range(1, NPAIR):
            nc.sync.dma_start(out=xp[i], in_=xT_d[:, 2 * i:2 * i + 2])

        wq_sb = []
        for h in range(H):
            t_ = p_wq.tile([128, NDT, 2, 128], f8e4, tag="wq", name=f"wq{h}")
            wq_sb.append(t_)
        nc.sync.dma_start(out=wq_sb[0], in_=wq_d[0])
        nc.sync.dma_start(out=wq_sb[1], in_=wq_d[1])

        cos_sb = p_const.tile([ROPE_DIMS, NK], bf16, tag="cos")
        nc.sync.dma_start(out=cos_sb, in_=cos_d[:])
        sin_sb = p_const.tile([ROPE_DIMS, NK], bf16, tag="sin")
        nc.sync.dma_start(out=sin_sb, in_=sin_d[:])

        msk_sb = p_const.tile([128, NQT, NSIG, 128], bf16, tag="msk")
        nc.sync.dma_start(out=msk_sb, in_=msk_d[:])

        # ---- K^T (RoPE'd) and V projections, dt-outer so PE starts on the
        # first xT tile while the rest stream in; head-0 q-projection is
        # folded in before the PSUM pool swap so PE never drains ----
        kt = p_kv.tile([128, NK], bf16, tag="kt")
        vt_sb = p_kv.tile([128, NK], bf16, tag="vt")
        v_sb = []
        for s in range(NKT):
            t_ = p_kv.tile([128, HD], bf16, tag=f"v{s}")
            v_sb.append(t_)
        qt_sb = []
        for h in range(H):
            t_ = p_qt.tile([128, CHUNK], bf16, tag="qt", name=f"qt{h}")
            qt_sb.append(t_)

        def q_proj(ps_qp, h):
            psq = ps_qp.tile([128, CHUNK], fp32, tag="ps_q")
            for p_ in range(NPAIR):
                nc.tensor.matmul(
                    psq, wq_sb[h][:, 2 * p_:2 * p_ + 2, 1, :],
                    xp[p_][:, :, 0, CHUNK:NK],
                    start=(p_ == 0), stop=False, perf_mode=DR,
                )
            for dt in range(NDT):
                p_, s_ = divmod(dt, 2)
                nc.tensor.matmul(
                    psq, wq_sb[h][:, dt, :, :], xp[p_][:, s_, :, CHUNK:NK],
                    start=False, stop=(dt == NDT - 1), perf_mode=DR,
                )
            _rope(p_tmp, qt_sb[h], psq, cos_sb[:, CHUNK:NK], sin_sb[:, CHUNK:NK])

        with tc.tile_pool(name="ps_q", bufs=2, space=bass.MemorySpace.PSUM) as ps_qp:
            with tc.tile_pool(
                name="ps_kv", bufs=1, space=bass.MemorySpace.PSUM
            ) as ps_kv:
                ps_k = [
                    ps_kv.tile([128, 512], fp32, tag=f"ps_k{i}", name=f"ps_k{i}")
                    for i in range(2)
                ]
                # V is computed transposed (one PSUM accumulation group per
                # bank — concurrent groups within a bank are illegal) and
                # tile-transposed to [pos, dv] via the DMA xbar afterwards
                ps_vt = [
                    ps_kv.tile([128, 512], fp32, tag=f"ps_vt{i}", name=f"ps_vt{i}")
                    for i in range(2)
                ]
                for p_ in range(NPAIR):
                    st = p_ == 0
                    for nh in range(2):
                        cols = slice(nh * 512, (nh + 1) * 512)
                        nc.tensor.matmul(
                            ps_k[nh], wk_sb[:, 2 * p_:2 * p_ + 2, 1, :],
                            xp[p_][:, :, 0, cols], start=st, stop=False,
                            perf_mode=DR,
                        )
                        nc.tensor.matmul(
                            ps_vt[nh], wv_sb[:, 2 * p_:2 * p_ + 2, 1, :],
                            xp[p_][:, :, 0, cols], start=st, stop=False,
                            perf_mode=DR,
                        )
                for dt in range(NDT):
                    p_, s_ = divmod(dt, 2)
                    sp = dt == NDT - 1
                    for nh in range(2):
                        cols = slice(nh * 512, (nh + 1) * 512)
                        nc.tensor.matmul(
                            ps_k[nh], wk_sb[:, dt, :, :],
                            xp[p_][:, s_, :, cols], start=False, stop=sp,
                            perf_mode=DR,
                        )
                        nc.tensor.matmul(
                            ps_vt[nh], wv_sb[:, dt, :, :],
                            xp[p_][:, s_, :, cols], start=False, stop=sp,
                            perf_mode=DR,
                        )
                # head-0 q-projection keeps PE busy while K-rope / V copies
                # drain the kv PSUM tiles
                q_proj(ps_qp, 0)
                for nh in range(2):
                    cols = slice(nh * 512, (nh + 1) * 512)
                    _rope(p_tmp, kt[:, cols], ps_k[nh],
                          cos_sb[:, cols], sin_sb[:, cols])
                    nc.vector.tensor_copy(vt_sb[:, cols], ps_vt[nh])
                for c in range(NKT):
                    nc.sync.dma_start_transpose(
                        out=v_sb[c], in_=vt_sb[:, c * 128:(c + 1) * 128]
                    )

            # ---- per-head attention (software-pipelined PE stream) ----
            otn = []
            for h in range(H):
                t_ = p_otn.tile([128, CHUNK], bf16, tag=f"otn{h}")
                otn.append(t_)

            with (
                tc.tile_pool(name="ps_s", bufs=2, space=bass.MemorySpace.PSUM) as ps_sp,
                tc.tile_pool(name="ps_o", bufs=2, space=bass.MemorySpace.PSUM) as ps_op,
            ):
                bias_sb = p_const.tile([128, D], bf16, tag="bias")
                wo_tiles = []
                for n in range(DN):
                    t_ = p_wo.tile([128, H, 512], bf16, tag="wo", name=f"wo{n}")
                    wo_tiles.append(t_)

                for h in range(H):
                    if h == 12:
                        # prefetch o-proj operands so the tail phase starts hot
                        nc.sync.dma_start(
                            out=bias_sb,
                            in_=bass.AP(tensor=bo_d, offset=0, ap=[[0, 128], [1, D]]),
                        )
                    if h == 14:
                        nc.sync.dma_start(out=wo_tiles[0], in_=wo_d[0])
                    qt = qt_sb[h]
                    otp = ps_op.tile([128, CHUNK], fp32, tag="ps_o")
                    dn = p_dn.tile([128, NQT, 128], fp32, tag="dn")
                    ess = []

                    def attn_unit(t):
                        pss = ps_sp.tile([128, NSIG, 128], fp32, tag="ps_s")
                        qsl = qt[:, t * 128:(t + 1) * 128]
                        for sig in range(NSIG):
                            s = t + sig
                            nc.tensor.matmul(
                                pss[:, sig, :], kt[:, s * 128:(s + 1) * 128], qsl,
                                start=True, stop=True,
                            )
                        es = p_es.tile([128, NSIG, 128], bf16, tag="es")
                        ess.append(es)
                        # q and k both carry the 2^15 fp8 psum factor; the exp
                        # scale divides it back out
                        nc.scalar.activation(es, pss, AF.Exp, scale=SCALE / (S2 * S2))
                        nc.vector.tensor_mul(es, es, msk_sb[:, t, :, :])
                        # fp16 add-tree: 2-byte packed DVE ops run 2x; exact
                        # zeros from the mask keep the boundary cores exact
                        r2 = p_red.tile([128, 2, 128], fp16, tag="r2")
                        nc.gpsimd.tensor_add(r2, es[:, 0:2, :], es[:, 2:4, :])
                        r1 = p_red.tile([128, 128], fp16, tag="r1")
                        nc.vector.tensor_add(r1, r2[:, 0, :], r2[:, 1, :])
                        red = p_red.tile([128, 128], fp16, tag="red")
                        nc.vector.tensor_add(red, r1, es[:, 4, :])
                        nc.gpsimd.partition_all_reduce(
                            dn[:, t, :], red, channels=128,
                            reduce_op=bass_isa.ReduceOp.add,
                        )

                    last_head = h == H - 1
                    pso_br = []
                    if last_head:
                        # the next-head q-projection no longer exists to cover
                        # the exp/mask chain; instead accumulate heads 0..14 of
                        # the first two o-proj units into the (free) ps_q
                        # slots, leaving the groups open until otn[15] lands
                        for i in range(2):
                            t_ = ps_qp.tile(
                                [128, CHUNK], fp32, tag="ps_q", name=f"ps_br{i}"
                            )
                            pso_br.append(t_)

                    def oproj_partial(i, hs):
                        for h2 in hs:
                            nc.tensor.matmul(
                                pso_br[i][:, 0:512],
                                otn[h2][:, i * 128:(i + 1) * 128],
                                wo_tiles[0][:, h2, :],
                                start=(h2 == 0), stop=(h2 == H - 1),
                            )

                    if h + 2 < H:
                        nc.sync.dma_start(out=wq_sb[h + 2], in_=wq_d[h + 2])
                    for t in range(2):
                        attn_unit(t)
                    if h + 1 < H:
                        q_proj(ps_qp, h + 1)
                    else:
                        oproj_partial(0, range(H - 1))
                    for t in range(2, NQT):
                        attn_unit(t)
                    if last_head:
                        oproj_partial(1, range(H - 1))
                    for t in range(NQT):
                        for sig in range(NSIG):
                            nc.tensor.matmul(
                                otp[:, t * 128:(t + 1) * 128],
                                v_sb[t + sig], ess[t][:, sig, :],
                                start=(sig == 0), stop=(sig == NSIG - 1),
                            )
                    rview = dn.rearrange("p t q -> p (t q)")
                    nc.vector.reciprocal(rview, rview)
                    nc.vector.tensor_mul(otn[h], otp, rview)
                    if last_head:
                        for i in range(2):
                            oproj_partial(i, [H - 1])
                            ob = p_ob.tile([128, 512], bf16, tag="ob")
                            nc.vector.tensor_add(
                                ob, pso_br[i], bias_sb[:, 0:512]
                            )
                            nc.scalar.dma_start(
                                out=out_d[i * 128:(i + 1) * 128, 0:512], in_=ob
                            )

                def oproj_slice(pso, n, t, hs):
                    for h in range(H):
                        nc.tensor.matmul(
                            pso[:, hs],
                            otn[h][:, t * 128:(t + 1) * 128],
                            wo_tiles[n][:, h, hs],
                            start=(h == 0), stop=(h == H - 1),
                        )
                    ob = p_ob.tile([128, 512], bf16, tag="ob")
                    nc.vector.tensor_add(
                        ob[:, hs], pso[:, hs],
                        bias_sb[:, n * 512 + hs.start:n * 512 + hs.stop],
                    )
                    nc.scalar.dma_start(
                        out=out_d[
                            t * 128:(t + 1) * 128,
                            n * 512 + hs.start:n * 512 + hs.stop,
                        ],
                        in_=ob[:, hs],
                    )

                def oproj_unit(pso, n, t):
                    oproj_slice(pso, n, t, slice(0, 512))

                # ---- o-projection + bias, straight out of the (now idle)
                # score-PSUM banks — no pool transition barrier ----
                nc.sync.dma_start(out=wo_tiles[1], in_=wo_d[1])
                for t in range(2, NQT):
                    pst = ps_sp.tile([128, NSIG, 128], fp32, tag="ps_s")
                    pso = pst.rearrange("p s q -> p (s q)")[:, 0:512]
                    oproj_unit(pso, 0, t)
                for n in range(1, DN):
                    if n + 1 < DN:
                        nc.sync.dma_start(out=wo_tiles[n + 1], in_=wo_d[n + 1])
                    for t in range(NQT):
                        pst = ps_sp.tile([128, NSIG, 128], fp32, tag="ps_s")
                        pso = pst.rearrange("p s q -> p (s q)")[:, 0:512]
                        if n == DN - 1 and t == NQT - 1:
                            # final unit: the last slice goes in the OTHER
                            # ps_s slot so its matmuls don't wait for the
                            # first slice's bias-add read (same-tile hazard),
                            # and the trailing store chain is short
                            oproj_slice(pso, n, t, slice(0, 384))
                            pst2 = ps_sp.tile(
                                [128, NSIG, 128], fp32, tag="ps_s", name="pst2"
                            )
                            pso2 = pst2.rearrange("p s q -> p (s q)")[:, 0:512]
                            oproj_slice(pso2, n, t, slice(384, 512))
                        else:
                            oproj_unit(pso, n, t)

    nc.compile()
    return nc


def _get_program():
    global _PROGRAM
    if _PROGRAM is None:
        _PROGRAM = _build_program()
    return _PROGRAM


def _q8pair(a, s):
    """fp8 e4m3 (hi, lo) pair of a*s; lo is quantized at the same scale."""
    hi = (a * s).astype(F8)
    lo = (a * s - hi.astype(np.float32)).astype(F8)
    return hi, lo


def _make_in_maps(x, Wq, Wk, Wv, Wo, bo):
    # host pre-layouts that mirror the SBUF tiles exactly (partition-major,
    # >=512B contiguous per partition) so every DMA runs at full bus rate.
    # q/k/v weights ship as fp8 (lo, hi) pairs in [p, dt, hl, col] layout.
    def wpair(W, ncol):
        hi, lo = _q8pair(np.asarray(W, np.float32), SW)
        # [hl, dt, p, col] -> [p, dt, hl, col]
        st = np.stack([lo, hi]).reshape(2, NDT, 128, ncol)
        return np.ascontiguousarray(st.transpose(2, 1, 0, 3))

    Wq_b = np.ascontiguousarray(
        wpair(Wq, D).reshape(128, NDT, 2, H, 128).transpose(3, 0, 1, 2, 4)
    )
    Wk_b = wpair(Wk, HD)
    Wv_b = wpair(Wv, HD)
    Wo_b = np.ascontiguousarray(
        np.asarray(Wo, np.float32).reshape(H, 128, DN, 512).transpose(2, 1, 0, 3)
    ).astype(BF16)
    bo_f = np.ascontiguousarray(
        np.asarray(bo, np.float32).reshape(1, D) * S2
    ).astype(BF16)

    inv_freq = np.exp(
        -np.log(np.float32(ROPE_BASE))
        * (np.arange(0, ROPE_DIMS, 2, dtype=np.float32) / np.float32(ROPE_DIMS))
    ).astype(np.float32)

    in_maps = []
    for c in range(8):
        b, g = divmod(c, 4)
        k_start = 512 * g - 512
        xs = np.zeros((NK, D), np.float32)
        lo = max(0, k_start)
        xs[lo - k_start:] = x[b, lo:k_start + NK]
        xhi, xlo = _q8pair(xs.T, SX)
        # [hl, dt, p, pos] -> [p, dt, hl, pos]
        xT = np.ascontiguousarray(
            np.stack([xhi, xlo]).reshape(2, NDT, 128, NK).transpose(2, 1, 0, 3)
        )

        pos = (k_start + np.arange(NK)).astype(np.float32)
        theta = pos[None, :] * inv_freq[:, None]          # [32, NK]
        cos2 = np.ascontiguousarray(
            np.concatenate([np.cos(theta)] * 2, axis=0)).astype(BF16)
        sin2 = np.ascontiguousarray(
            np.concatenate([-np.sin(theta), np.sin(theta)], axis=0)).astype(BF16)

        m = np.zeros((NQT, NSIG, 128, 128), np.float32)
        for t in range(NQT):
            Tg = NQT * g + t
            for sig in range(NSIG):
                S = Tg - 4 + sig
                if S < 0:
                    continue
                i = (128 * Tg + np.arange(128))[None, :]   # queries (cols)
                j = (128 * S + np.arange(128))[:, None]    # keys (rows)
                m[t, sig] = (((i - j) >= 0) & ((i - j) < WINDOW)).astype(np.float32)
        # SBUF layout [k, t, sig, q]
        masks = np.ascontiguousarray(m.transpose(2, 0, 1, 3)).astype(BF16)

        in_maps.append({
            "xT8": xT, "Wq": Wq_b, "Wk": Wk_b, "Wv": Wv_b, "Wo": Wo_b,
            "bo": bo_f, "cosT": cos2, "sinT": sin2, "masks": masks,
        })
    return in_maps


def _unshard(results):
    out = np.zeros((B, L, D), np.float32)
    for c in range(8):
        b, g = divmod(c, 4)
        # the fp8 quantization scales (2^15) ride through the whole pipeline
        out[b, CHUNK * g:CHUNK * (g + 1)] = results[c]["out"].astype(np.float32) / S2
    return out


def kernel(x, Wq, Wk, Wv, Wo, bo):
    from concourse.bass_utils import run_bass_kernel_spmd

    nc = _get_program()
    in_maps = _make_in_maps(x, Wq, Wk, Wv, Wo, bo)
    res = run_bass_kernel_spmd(nc, in_maps, core_ids=list(range(8)))
    return _unshard(res.results)



# revision 25
# speedup vs baseline: 1.0426x; 1.0226x over previous
"""Local sliding-window attention block (MQA + partial RoPE) on 8 TRN2 cores.

Sharding: 2 batches x 4 sequence chunks of 512 queries each. Each core
computes q/k/v projections for its chunk (keys include a 512-token halo),
windowed attention (window=512, causal), and the o-projection for its own
query rows — so the host-side unshard is a pure concatenation.

On-chip layout: everything transposed (feature dim on partitions).
  xT[d, pos]  ->  Q^T[dh, q] / K^T[dh, k] (RoPE'd)  ->  S^T[k, q]
  -> exp -> P^T[k, q] (bf16, multiplicative 0/1 masks)
  -> O^T[dv, q] = V.T-matmul  -> normalized by softmax denominators
     (partition_all_reduce on GPSIMD)  -> used directly as lhsT of o-proj.
All matmuls bf16 inputs, fp32 PSUM accumulation.

Schedule notes (tuned against the CoreSim cost model):
 - weight/const DMAs are host-pre-laid-out to match SBUF (>=512B rows,
   full DMA bus rate) and ordered so the K/V projection starts on the
   first xT tile; per-head Wq streams 2 heads ahead with 3 buffers.
 - V is projected transposed (one PSUM accumulation group per bank) and
   flipped to [pos, dv] tiles with DMA xbar transposes.
 - per head the PE stream is software-pipelined: scores(t0,t1) -> next
   head's q-projection -> scores(t2,t3) -> all AV matmuls, so the
   exp/mask chain on Act/DVE hides under the q-projection. The last
   head interleaves heads 0..14 of the first two o-proj units instead.
 - softmax sig-reduction is an fp16 add-tree (2-byte packed DVE ops run
   2x, first add on GPSIMD) instead of a strided TensorReduce.
 - rope half-swaps ride the GPSIMD SWDGE queue so the SP load queue is
   never blocked behind data-dependent transfers.
 - the whole o-projection runs out of the score-PSUM banks (no pool
   transition barrier); the final unit is split so the trailing
   bias-add + store latency shrinks.
"""

import numpy as np
import ml_dtypes

BF16 = ml_dtypes.bfloat16
F8 = ml_dtypes.float8_e4m3

B, L, D = 2, 2048, 2048
H, HD = 16, 128
ROPE_DIMS, HALF = 64, 32
WINDOW = 512
ROPE_BASE = 10000.0
SCALE = HD ** -0.5
# fp8 quantization scales: x is quantized at 16x, weights at 2048x, so every
# projection PSUM carries a 2^15 factor that the exp-scale / host descale absorb
SX = 16.0
SW = 2048.0
S2 = SX * SW  # 2^15
# attention outputs are quantized fp8 at 64x (the V psum->sbuf copy descales by
# SO/S2 so the AV output lands at 64*true already); Wo fp8 at 2048x
SO = 64.0
SWO = 2048.0

CHUNK = 512            # queries per core
NK = 1024              # keys (incl. halo) per core
NQT = CHUNK // 128     # 4 local query tiles
NKT = NK // 128        # 8 local key tiles
NSIG = 5               # key tiles in window per query tile
NDT = D // 128         # 16 contraction tiles over embedding dim
NPAIR = NDT // 2       # 8 dt pairs (256-deep DoubleRow contraction units)
DN = D // 512          # 4 o-proj column blocks

_PROGRAM = None


def _build_program():
    from contextlib import ExitStack
    import concourse.bass as bass
    import concourse.mybir as mybir
    import concourse.tile as tile
    import concourse.bass_isa as bass_isa
    from concourse import bacc

    fp32 = mybir.dt.float32
    fp16 = mybir.dt.float16
    bf16 = mybir.dt.bfloat16
    f8e4 = mybir.dt.float8e4
    DR = mybir.MatmulPerfMode.DoubleRow
    AF = mybir.ActivationFunctionType

    nc = bacc.Bacc(None, target_bir_lowering=False)

    # fp8 operands carry (hi, lo) compensation pairs: x tiles are laid out
    # [p, dt, (hi, lo), pos], weight tiles [p, dt, (lo, hi), col] so that a
    # single DoubleRow matmul over the hl axis yields the Wl.T@Xh + Wh.T@Xl
    # cross terms, and a DoubleRow over a dt pair at hl=hi yields the main
    # term with a 256-deep contraction at half the per-row cost.
    xT_d = nc.dram_tensor("xT8", [128, NDT, 2, NK], f8e4, kind="ExternalInput")
    wq_d = nc.dram_tensor("Wq", [H, 128, NDT, 2, 128], f8e4, kind="ExternalInput")
    wk_d = nc.dram_tensor("Wk", [128, NDT, 2, HD], f8e4, kind="ExternalInput")
    wv_d = nc.dram_tensor("Wv", [128, NDT, 2, HD], f8e4, kind="ExternalInput")
    wo_d = nc.dram_tensor("Wo", [DN, 128, H, 2, 512], f8e4, kind="ExternalInput")
    bo_d = nc.dram_tensor("bo", [1, D], bf16, kind="ExternalInput")
    cos_d = nc.dram_tensor("cosT", [ROPE_DIMS, NK], bf16, kind="ExternalInput")
    sin_d = nc.dram_tensor("sinT", [ROPE_DIMS, NK], bf16, kind="ExternalInput")
    msk_d = nc.dram_tensor("masks", [128, NQT, NSIG, 128], bf16, kind="ExternalInput")
    out_d = nc.dram_tensor("out", [CHUNK, D], bf16, kind="ExternalOutput")

    def _rope(pool, out_bf, ps, cos2, sin2m, eng=None):
        """out[0:64] = rotary(ps[0:64]); out[64:128] = ps[64:128].

        ps fp32 PSUM, out bf16. cos2/sin2m bf16 [64, n] row tables
        (rows [0:32]==[32:64]==cos; sin rows [0:32]=-sin, [32:64]=+sin).
        The half-swap goes through two partition-shifting DMAs; the
        PSUM->SBUF casts run on Act, the bf16 elementwise math runs on
        DVE in the 2x packed-16-bit mode.
        """
        n = cos2.shape[-1]
        sb64 = pool.tile([ROPE_DIMS, n], bf16, tag="rope_sb64")
        nc.scalar.copy(sb64, ps[0:ROPE_DIMS])
        ss = pool.tile([ROPE_DIMS, n], bf16, tag="rope_ss")
        # SWDGE (gpsimd) queue: keeps these data-dependent shuffles out of
        # the SP load queue so weight streaming is never blocked behind them
        nc.gpsimd.dma_start(out=ss[0:HALF], in_=sb64[HALF:ROPE_DIMS])
        nc.gpsimd.dma_start(out=ss[HALF:ROPE_DIMS], in_=sb64[0:HALF])
        t1 = pool.tile([ROPE_DIMS, n], bf16, tag="rope_t1")
        eng = eng or nc.vector
        eng.tensor_mul(t1, sb64, cos2)
        eng.tensor_mul(ss, ss, sin2m)
        eng.tensor_add(out_bf[0:ROPE_DIMS], t1, ss)
        nc.scalar.copy(out_bf[ROPE_DIMS:HD], ps[ROPE_DIMS:HD])

    with tile.TileContext(nc) as tc, ExitStack() as ctx:
        p_const = ctx.enter_context(tc.tile_pool(name="const", bufs=1))
        p_xt = ctx.enter_context(tc.tile_pool(name="xt", bufs=1))
        p_kv = ctx.enter_context(tc.tile_pool(name="kv", bufs=1))
        p_wq = ctx.enter_context(tc.tile_pool(name="wq", bufs=3))
        p_qt = ctx.enter_context(tc.tile_pool(name="qt", bufs=3))
        p_es = ctx.enter_context(tc.tile_pool(name="es", bufs=6))
        p_red = ctx.enter_context(tc.tile_pool(name="red", bufs=8))
        p_dn = ctx.enter_context(tc.tile_pool(name="dn", bufs=2))
        p_tmp = ctx.enter_context(tc.tile_pool(name="tmp", bufs=4))
        p_otn = ctx.enter_context(tc.tile_pool(name="otn", bufs=1))
        p_wo = ctx.enter_context(tc.tile_pool(name="wo", bufs=2))
        p_ob = ctx.enter_context(tc.tile_pool(name="ob", bufs=4))

        # ---- loads, ordered so compute starts ASAP ----
        wk_sb = p_const.tile([128, NDT, 2, HD], f8e4, tag="wk")
        nc.sync.dma_start(out=wk_sb, in_=wk_d[:])

        # x in 8 dt-pair tiles [p, 2(dt), 2(hl), pos] so the first DoubleRow
        # starts after the first pair lands
        xp = []
        for i in range(NPAIR):
            t_ = p_xt.tile([128, 2, 2, NK], f8e4, tag=f"xp{i}")
            xp.append(t_)
        nc.sync.dma_start(out=xp[0], in_=xT_d[:, 0:2])

        wv_sb = p_const.tile([128, NDT, 2, HD], f8e4, tag="wv")
        nc.sync.dma_start(out=wv_sb, in_=wv_d[:])
        for i in range(1, NPAIR):
            nc.sync.dma_start(out=xp[i], in_=xT_d[:, 2 * i:2 * i + 2])

        wq_sb = []
        for h in range(H):
            t_ = p_wq.tile([128, NDT, 2, 128], f8e4, tag="wq", name=f"wq{h}")
            wq_sb.append(t_)
        nc.sync.dma_start(out=wq_sb[0], in_=wq_d[0])
        nc.sync.dma_start(out=wq_sb[1], in_=wq_d[1])

        cos_sb = p_const.tile([ROPE_DIMS, NK], bf16, tag="cos")
        nc.sync.dma_start(out=cos_sb, in_=cos_d[:])
        sin_sb = p_const.tile([ROPE_DIMS, NK], bf16, tag="sin")
        nc.sync.dma_start(out=sin_sb, in_=sin_d[:])

        msk_sb = p_const.tile([128, NQT, NSIG, 128], bf16, tag="msk")
        nc.sync.dma_start(out=msk_sb, in_=msk_d[:])

        # ---- K^T (RoPE'd) and V projections, dt-outer so PE starts on the
        # first xT tile while the rest stream in; head-0 q-projection is
        # folded in before the PSUM pool swap so PE never drains ----
        kt = p_kv.tile([128, NK], bf16, tag="kt")
        vt_sb = p_kv.tile([128, NK], bf16, tag="vt")
        v_sb = []
        for s in range(NKT):
            t_ = p_kv.tile([128, HD], bf16, tag=f"v{s}")
            v_sb.append(t_)
        qt_sb = []
        for h in range(H):
            t_ = p_qt.tile([128, CHUNK], bf16, tag="qt", name=f"qt{h}")
            qt_sb.append(t_)

        def q_proj(ps_qp, h):
            psq = ps_qp.tile([128, CHUNK], fp32, tag="ps_q")
            for p_ in range(NPAIR):
                nc.tensor.matmul(
                    psq, wq_sb[h][:, 2 * p_:2 * p_ + 2, 1, :],
                    xp[p_][:, :, 0, CHUNK:NK],
                    start=(p_ == 0), stop=False, perf_mode=DR,
                )
            for dt in range(NDT):
                p_, s_ = divmod(dt, 2)
                nc.tensor.matmul(
                    psq, wq_sb[h][:, dt, :, :], xp[p_][:, s_, :, CHUNK:NK],
                    start=False, stop=(dt == NDT - 1), perf_mode=DR,
                )
            _rope(p_tmp, qt_sb[h], psq, cos_sb[:, CHUNK:NK], sin_sb[:, CHUNK:NK],
                  eng=nc.gpsimd)

        with tc.tile_pool(name="ps_q", bufs=2, space=bass.MemorySpace.PSUM) as ps_qp:
            with tc.tile_pool(
                name="ps_kv", bufs=1, space=bass.MemorySpace.PSUM
            ) as ps_kv:
                ps_k = [
                    ps_kv.tile([128, 512], fp32, tag=f"ps_k{i}", name=f"ps_k{i}")
                    for i in range(2)
                ]
                # V is computed transposed (one PSUM accumulation group per
                # bank — concurrent groups within a bank are illegal) and
                # tile-transposed to [pos, dv] via the DMA xbar afterwards
                ps_vt = [
                    ps_kv.tile([128, 512], fp32, tag=f"ps_vt{i}", name=f"ps_vt{i}")
                    for i in range(2)
                ]
                for p_ in range(NPAIR):
                    st = p_ == 0
                    for nh in range(2):
                        cols = slice(nh * 512, (nh + 1) * 512)
                        nc.tensor.matmul(
                            ps_k[nh], wk_sb[:, 2 * p_:2 * p_ + 2, 1, :],
                            xp[p_][:, :, 0, cols], start=st, stop=False,
                            perf_mode=DR,
                        )
                        nc.tensor.matmul(
                            ps_vt[nh], wv_sb[:, 2 * p_:2 * p_ + 2, 1, :],
                            xp[p_][:, :, 0, cols], start=st, stop=False,
                            perf_mode=DR,
                        )
                for dt in range(NDT):
                    p_, s_ = divmod(dt, 2)
                    sp = dt == NDT - 1
                    for nh in range(2):
                        cols = slice(nh * 512, (nh + 1) * 512)
                        nc.tensor.matmul(
                            ps_k[nh], wk_sb[:, dt, :, :],
                            xp[p_][:, s_, :, cols], start=False, stop=sp,
                            perf_mode=DR,
                        )
                        nc.tensor.matmul(
                            ps_vt[nh], wv_sb[:, dt, :, :],
                            xp[p_][:, s_, :, cols], start=False, stop=sp,
                            perf_mode=DR,
                        )
                # head-0 q-projection keeps PE busy while K-rope / V copies
                # drain the kv PSUM tiles
                q_proj(ps_qp, 0)
                for nh in range(2):
                    cols = slice(nh * 512, (nh + 1) * 512)
                    _rope(p_tmp, kt[:, cols], ps_k[nh],
                          cos_sb[:, cols], sin_sb[:, cols])
                    # scaled copy on Act: divides out the 2^15 fp8 psum factor
                    # and bakes in the SO=64 scale the otn fp8 quantization
                    # wants, so the attention-output path needs no extra ops
                    nc.scalar.activation(
                        vt_sb[:, cols], ps_vt[nh], AF.Copy, scale=SO / S2
                    )
                for c in range(NKT):
                    nc.sync.dma_start_transpose(
                        out=v_sb[c], in_=vt_sb[:, c * 128:(c + 1) * 128]
                    )

            # ---- per-head attention (software-pipelined PE stream) ----
            # attention outputs in fp8 (hi, lo) head-pair tiles, ready to be
            # DoubleRow o-proj operands: [dv, 2(head-in-pair), 2(hi/lo), q]
            otn_p = []
            for j in range(H // 2):
                t_ = p_otn.tile([128, 2, 2, CHUNK], f8e4, tag=f"otnp{j}")
                otn_p.append(t_)

            with (
                tc.tile_pool(name="ps_s", bufs=2, space=bass.MemorySpace.PSUM) as ps_sp,
                tc.tile_pool(name="ps_o", bufs=2, space=bass.MemorySpace.PSUM) as ps_op,
            ):
                bias_sb = p_const.tile([128, D], bf16, tag="bias")
                wo_tiles = []
                for n in range(DN):
                    t_ = p_wo.tile([128, H, 2, 512], f8e4, tag="wo", name=f"wo{n}")
                    wo_tiles.append(t_)

                for h in range(H):
                    if h == 12:
                        # prefetch o-proj operands so the tail phase starts hot
                        nc.sync.dma_start(
                            out=bias_sb,
                            in_=bass.AP(tensor=bo_d, offset=0, ap=[[0, 128], [1, D]]),
                        )
                    if h == 14:
                        nc.sync.dma_start(out=wo_tiles[0], in_=wo_d[0])
                    qt = qt_sb[h]
                    otp = ps_op.tile([128, CHUNK], fp32, tag="ps_o")
                    dn = p_dn.tile([128, NQT, 128], fp32, tag="dn")
                    ess = []

                    def attn_unit(t):
                        pss = ps_sp.tile([128, NSIG, 128], fp32, tag="ps_s")
                        qsl = qt[:, t * 128:(t + 1) * 128]
                        for sig in range(NSIG):
                            s = t + sig
                            nc.tensor.matmul(
                                pss[:, sig, :], kt[:, s * 128:(s + 1) * 128], qsl,
                                start=True, stop=True,
                            )
                        es = p_es.tile([128, NSIG, 128], bf16, tag="es")
                        ess.append(es)
                        # q and k both carry the 2^15 fp8 psum factor; the exp
                        # scale divides it back out
                        nc.scalar.activation(es, pss, AF.Exp, scale=SCALE / (S2 * S2))
                        nc.vector.tensor_mul(es, es, msk_sb[:, t, :, :])
                        # fp16 add-tree: 2-byte packed DVE ops run 2x; exact
                        # zeros from the mask keep the boundary cores exact
                        r2 = p_red.tile([128, 2, 128], fp16, tag="r2")
                        nc.gpsimd.tensor_add(r2, es[:, 0:2, :], es[:, 2:4, :])
                        r1 = p_red.tile([128, 128], fp16, tag="r1")
                        nc.vector.tensor_add(r1, r2[:, 0, :], r2[:, 1, :])
                        red = p_red.tile([128, 128], fp16, tag="red")
                        nc.vector.tensor_add(red, r1, es[:, 4, :])
                        nc.gpsimd.partition_all_reduce(
                            dn[:, t, :], red, channels=128,
                            reduce_op=bass_isa.ReduceOp.add,
                        )

                    last_head = h == H - 1
                    pso_br = []
                    if last_head:
                        # the next-head q-projection no longer exists to cover
                        # the exp/mask chain; instead accumulate heads 0..14 of
                        # the first two o-proj units into the (free) ps_q
                        # slots, leaving the groups open until otn[15] lands
                        for i in range(2):
                            t_ = ps_qp.tile(
                                [128, CHUNK], fp32, tag="ps_q", name=f"ps_br{i}"
                            )
                            pso_br.append(t_)

                    def oproj_mains(pso, n, tq, hs, js, start):
                        for j in js:
                            nc.tensor.matmul(
                                pso, otn_p[j][:, :, 0, tq],
                                wo_tiles[n][:, 2 * j:2 * j + 2, 1, hs],
                                start=(start and j == js[0]), stop=False,
                                perf_mode=DR,
                            )

                    def oproj_corrs(pso, n, tq, hs, h2s, stop):
                        for h2 in h2s:
                            j, s_ = divmod(h2, 2)
                            nc.tensor.matmul(
                                pso, otn_p[j][:, s_, :, tq],
                                wo_tiles[n][:, h2, :, hs],
                                start=False, stop=(stop and h2 == h2s[-1]),
                                perf_mode=DR,
                            )

                    def oproj_partial(i, part):
                        tq = slice(i * 128, (i + 1) * 128)
                        pso = pso_br[i][:, 0:512]
                        if part == 0:
                            # everything not touching head 15 (pair 7 main)
                            oproj_mains(pso, 0, tq, slice(0, 512),
                                        list(range(7)), True)
                            oproj_corrs(pso, 0, tq, slice(0, 512),
                                        list(range(15)), False)
                        else:
                            oproj_mains(pso, 0, tq, slice(0, 512), [7], False)
                            oproj_corrs(pso, 0, tq, slice(0, 512), [15], True)

                    if h + 2 < H:
                        nc.sync.dma_start(out=wq_sb[h + 2], in_=wq_d[h + 2])
                    for t in range(2):
                        attn_unit(t)
                    if h + 1 < H:
                        q_proj(ps_qp, h + 1)
                    else:
                        oproj_partial(0, 0)
                    for t in range(2, NQT):
                        attn_unit(t)
                    if last_head:
                        oproj_partial(1, 0)
                    for t in range(NQT):
                        for sig in range(NSIG):
                            nc.tensor.matmul(
                                otp[:, t * 128:(t + 1) * 128],
                                v_sb[t + sig], ess[t][:, sig, :],
                                start=(sig == 0), stop=(sig == NSIG - 1),
                            )
                    rview = dn.rearrange("p t q -> p (t q)")
                    nc.vector.reciprocal(rview, rview)
                    # otn = 64 * normalized head output (vt pre-carried SO/S2);
                    # hi/lo fp8 extraction rides the Pool engine
                    o16 = p_tmp.tile([128, CHUNK], fp16, tag="o16")
                    nc.vector.tensor_mul(o16, otp, rview)
                    j_, s_ = divmod(h, 2)
                    nc.gpsimd.tensor_copy(otn_p[j_][:, s_, 0, :], o16)
                    nc.gpsimd.tensor_sub(
                        otn_p[j_][:, s_, 1, :], o16, otn_p[j_][:, s_, 0, :]
                    )
                    if last_head:
                        for i in range(2):
                            oproj_partial(i, 1)
                            ob = p_ob.tile([128, 512], bf16, tag="ob")
                            nc.vector.tensor_add(
                                ob, pso_br[i], bias_sb[:, 0:512]
                            )
                            nc.scalar.dma_start(
                                out=out_d[i * 128:(i + 1) * 128, 0:512], in_=ob
                            )

                def oproj_slice(pso, n, t, hs):
                    tq = slice(t * 128, (t + 1) * 128)
                    for j in range(H // 2):
                        nc.tensor.matmul(
                            pso[:, hs], otn_p[j][:, :, 0, tq],
                            wo_tiles[n][:, 2 * j:2 * j + 2, 1, hs],
                            start=(j == 0), stop=False, perf_mode=DR,
                        )
                    for h in range(H):
                        j, s_ = divmod(h, 2)
                        nc.tensor.matmul(
                            pso[:, hs], otn_p[j][:, s_, :, tq],
                            wo_tiles[n][:, h, :, hs],
                            start=False, stop=(h == H - 1), perf_mode=DR,
                        )
                    ob = p_ob.tile([128, 512], bf16, tag="ob")
                    nc.vector.tensor_add(
                        ob[:, hs], pso[:, hs],
                        bias_sb[:, n * 512 + hs.start:n * 512 + hs.stop],
                    )
                    nc.scalar.dma_start(
                        out=out_d[
                            t * 128:(t + 1) * 128,
                            n * 512 + hs.start:n * 512 + hs.stop,
                        ],
                        in_=ob[:, hs],
                    )

                def oproj_unit(pso, n, t):
                    oproj_slice(pso, n, t, slice(0, 512))

                # ---- o-projection + bias, straight out of the (now idle)
                # score-PSUM banks — no pool transition barrier ----
                nc.sync.dma_start(out=wo_tiles[1], in_=wo_d[1])
                for t in range(2, NQT):
                    pst = ps_sp.tile([128, NSIG, 128], fp32, tag="ps_s")
                    pso = pst.rearrange("p s q -> p (s q)")[:, 0:512]
                    oproj_unit(pso, 0, t)
                for n in range(1, DN):
                    if n + 1 < DN:
                        nc.sync.dma_start(out=wo_tiles[n + 1], in_=wo_d[n + 1])
                    for t in range(NQT):
                        pst = ps_sp.tile([128, NSIG, 128], fp32, tag="ps_s")
                        pso = pst.rearrange("p s q -> p (s q)")[:, 0:512]
                        if n == DN - 1 and t == NQT - 1:
                            # final unit: the last slice goes in the OTHER
                            # ps_s slot so its matmuls don't wait for the
                            # first slice's bias-add read (same-tile hazard),
                            # and the trailing store chain is short
                            oproj_slice(pso, n, t, slice(0, 384))
                            pst2 = ps_sp.tile(
                                [128, NSIG, 128], fp32, tag="ps_s", name="pst2"
                            )
                            pso2 = pst2.rearrange("p s q -> p (s q)")[:, 0:512]
                            oproj_slice(pso2, n, t, slice(384, 512))
                        else:
                            oproj_unit(pso, n, t)

    nc.compile()
    return nc


def _get_program():
    global _PROGRAM
    if _PROGRAM is None:
        _PROGRAM = _build_program()
    return _PROGRAM


def _q8pair(a, s):
    """fp8 e4m3 (hi, lo) pair of a*s; lo is quantized at the same scale."""
    hi = (a * s).astype(F8)
    lo = (a * s - hi.astype(np.float32)).astype(F8)
    return hi, lo


def _make_in_maps(x, Wq, Wk, Wv, Wo, bo):
    # host pre-layouts that mirror the SBUF tiles exactly (partition-major,
    # >=512B contiguous per partition) so every DMA runs at full bus rate.
    # q/k/v weights ship as fp8 (lo, hi) pairs in [p, dt, hl, col] layout.
    def wpair(W, ncol):
        hi, lo = _q8pair(np.asarray(W, np.float32), SW)
        # [hl, dt, p, col] -> [p, dt, hl, col]
        st = np.stack([lo, hi]).reshape(2, NDT, 128, ncol)
        return np.ascontiguousarray(st.transpose(2, 1, 0, 3))

    Wq_b = np.ascontiguousarray(
        wpair(Wq, D).reshape(128, NDT, 2, H, 128).transpose(3, 0, 1, 2, 4)
    )
    Wk_b = wpair(Wk, HD)
    Wv_b = wpair(Wv, HD)
    whi, wlo = _q8pair(np.asarray(Wo, np.float32), SWO)
    # [hl, h, p, n, c] -> [n, p, h, hl, c]
    Wo_b = np.ascontiguousarray(
        np.stack([wlo, whi]).reshape(2, H, 128, DN, 512).transpose(3, 2, 1, 0, 4)
    )
    bo_f = np.ascontiguousarray(
        np.asarray(bo, np.float32).reshape(1, D) * (SO * SWO)
    ).astype(BF16)

    inv_freq = np.exp(
        -np.log(np.float32(ROPE_BASE))
        * (np.arange(0, ROPE_DIMS, 2, dtype=np.float32) / np.float32(ROPE_DIMS))
    ).astype(np.float32)

    in_maps = []
    for c in range(8):
        b, g = divmod(c, 4)
        k_start = 512 * g - 512
        xs = np.zeros((NK, D), np.float32)
        lo = max(0, k_start)
        xs[lo - k_start:] = x[b, lo:k_start + NK]
        xhi, xlo = _q8pair(xs.T, SX)
        # [hl, dt, p, pos] -> [p, dt, hl, pos]
        xT = np.ascontiguousarray(
            np.stack([xhi, xlo]).reshape(2, NDT, 128, NK).transpose(2, 1, 0, 3)
        )

        pos = (k_start + np.arange(NK)).astype(np.float32)
        theta = pos[None, :] * inv_freq[:, None]          # [32, NK]
        cos2 = np.ascontiguousarray(
            np.concatenate([np.cos(theta)] * 2, axis=0)).astype(BF16)
        sin2 = np.ascontiguousarray(
            np.concatenate([-np.sin(theta), np.sin(theta)], axis=0)).astype(BF16)

        m = np.zeros((NQT, NSIG, 128, 128), np.float32)
        for t in range(NQT):
            Tg = NQT * g + t
            for sig in range(NSIG):
                S = Tg - 4 + sig
                if S < 0:
                    continue
                i = (128 * Tg + np.arange(128))[None, :]   # queries (cols)
                j = (128 * S + np.arange(128))[:, None]    # keys (rows)
                m[t, sig] = (((i - j) >= 0) & ((i - j) < WINDOW)).astype(np.float32)
        # SBUF layout [k, t, sig, q]
        masks = np.ascontiguousarray(m.transpose(2, 0, 1, 3)).astype(BF16)

        in_maps.append({
            "xT8": xT, "Wq": Wq_b, "Wk": Wk_b, "Wv": Wv_b, "Wo": Wo_b,
            "bo": bo_f, "cosT": cos2, "sinT": sin2, "masks": masks,
        })
    return in_maps


def _unshard(results):
    out = np.zeros((B, L, D), np.float32)
    for c in range(8):
        b, g = divmod(c, 4)
        # the o-proj psum carries the otn (64x) and Wo (2048x) fp8 scales
        out[b, CHUNK * g:CHUNK * (g + 1)] = (
            results[c]["out"].astype(np.float32) / (SO * SWO)
        )
    return out


def kernel(x, Wq, Wk, Wv, Wo, bo):
    from concourse.bass_utils import run_bass_kernel_spmd

    nc = _get_program()
    in_maps = _make_in_maps(x, Wq, Wk, Wv, Wo, bo)
    res = run_bass_kernel_spmd(nc, in_maps, core_ids=list(range(8)))
    return _unshard(res.results)



# revision 36
# speedup vs baseline: 1.0559x; 1.0127x over previous
"""Local sliding-window attention block (MQA + partial RoPE) on 8 TRN2 cores.

Sharding: 2 batches x 4 sequence chunks of 512 queries each. Each core
computes q/k/v projections for its chunk (keys include a 512-token halo),
windowed attention (window=512, causal), and the o-projection for its own
query rows — so the host-side unshard is a pure concatenation.

On-chip layout: everything transposed (feature dim on partitions).
  xT[d, pos]  ->  Q^T[dh, q] / K^T[dh, k] (RoPE'd)  ->  S^T[k, q]
  -> exp -> P^T[k, q] (bf16, multiplicative 0/1 masks)
  -> O^T[dv, q] = V.T-matmul  -> normalized by softmax denominators
     (partition_all_reduce on GPSIMD)  -> used directly as lhsT of o-proj.
All matmuls bf16 inputs, fp32 PSUM accumulation.

Schedule notes (tuned against the CoreSim cost model):
 - weight/const DMAs are host-pre-laid-out to match SBUF (>=512B rows,
   full DMA bus rate) and ordered so the K/V projection starts on the
   first xT tile; per-head Wq streams 2 heads ahead with 3 buffers.
 - V is projected transposed (one PSUM accumulation group per bank) and
   flipped to [pos, dv] tiles with DMA xbar transposes.
 - per head the PE stream is software-pipelined: scores(t0,t1) -> next
   head's q-projection -> scores(t2,t3) -> all AV matmuls, so the
   exp/mask chain on Act/DVE hides under the q-projection. The last
   head interleaves heads 0..14 of the first two o-proj units instead.
 - softmax sig-reduction is an fp16 add-tree (2-byte packed DVE ops run
   2x, first add on GPSIMD) instead of a strided TensorReduce.
 - rope half-swaps ride the GPSIMD SWDGE queue so the SP load queue is
   never blocked behind data-dependent transfers.
 - the whole o-projection runs out of the score-PSUM banks (no pool
   transition barrier); the final unit is split so the trailing
   bias-add + store latency shrinks.
"""

import numpy as np
import ml_dtypes

BF16 = ml_dtypes.bfloat16
F8 = ml_dtypes.float8_e4m3

B, L, D = 2, 2048, 2048
H, HD = 16, 128
ROPE_DIMS, HALF = 64, 32
WINDOW = 512
ROPE_BASE = 10000.0
SCALE = HD ** -0.5
# fp8 quantization scales: x is quantized at 16x, weights at 2048x, so every
# projection PSUM carries a 2^15 factor that the exp-scale / host descale absorb
SX = 16.0
SW = 2048.0
S2 = SX * SW  # 2^15
# attention outputs are quantized fp8 at 64x (the V psum->sbuf copy descales by
# SO/S2 so the AV output lands at 64*true already); Wo fp8 at 2048x
SO = 64.0
SWO = 2048.0

CHUNK = 512            # queries per core
NK = 1024              # keys (incl. halo) per core
NQT = CHUNK // 128     # 4 local query tiles
NKT = NK // 128        # 8 local key tiles
NSIG = 5               # key tiles in window per query tile
NDT = D // 128         # 16 contraction tiles over embedding dim
NPAIR = NDT // 2       # 8 dt pairs (256-deep DoubleRow contraction units)
DN = D // 512          # 4 o-proj column blocks

_PROGRAM = None


def _build_program():
    from contextlib import ExitStack
    import concourse.bass as bass
    import concourse.mybir as mybir
    import concourse.tile as tile
    import concourse.bass_isa as bass_isa
    from concourse import bacc

    fp32 = mybir.dt.float32
    fp16 = mybir.dt.float16
    bf16 = mybir.dt.bfloat16
    f8e4 = mybir.dt.float8e4
    DR = mybir.MatmulPerfMode.DoubleRow
    AF = mybir.ActivationFunctionType

    nc = bacc.Bacc(None, target_bir_lowering=False)

    # fp8 operands carry (hi, lo) compensation pairs: x tiles are laid out
    # [p, dt, (hi, lo), pos], weight tiles [p, dt, (lo, hi), col] so that a
    # single DoubleRow matmul over the hl axis yields the Wl.T@Xh + Wh.T@Xl
    # cross terms, and a DoubleRow over a dt pair at hl=hi yields the main
    # term with a 256-deep contraction at half the per-row cost.
    xT_d = nc.dram_tensor("xT8", [128, NDT, 2, NK], f8e4, kind="ExternalInput")
    wq_d = nc.dram_tensor("Wq", [H, 128, NDT, 2, 128], f8e4, kind="ExternalInput")
    wk_d = nc.dram_tensor("Wk", [128, NDT, 2, HD], f8e4, kind="ExternalInput")
    wv_d = nc.dram_tensor("Wv", [128, NDT, 2, HD], f8e4, kind="ExternalInput")
    wo_d = nc.dram_tensor("Wo", [DN, 128, H, 2, 512], f8e4, kind="ExternalInput")
    bo_d = nc.dram_tensor("bo", [1, D], bf16, kind="ExternalInput")
    cos_d = nc.dram_tensor("cosT", [ROPE_DIMS, NK], bf16, kind="ExternalInput")
    sin_d = nc.dram_tensor("sinT", [ROPE_DIMS, NK], bf16, kind="ExternalInput")
    msk_d = nc.dram_tensor("masks", [128, NQT, NSIG, 128], bf16, kind="ExternalInput")
    out_d = nc.dram_tensor("out", [CHUNK, D], bf16, kind="ExternalOutput")

    def _rope(pool, out_bf, ps, cos2, sin2m, eng=None):
        """out[0:64] = rotary(ps[0:64]); out[64:128] = ps[64:128].

        ps fp32 PSUM, out bf16. cos2/sin2m bf16 [64, n] row tables
        (rows [0:32]==[32:64]==cos; sin rows [0:32]=-sin, [32:64]=+sin).
        The half-swap goes through two partition-shifting DMAs; the
        PSUM->SBUF casts run on Act, the bf16 elementwise math runs on
        DVE in the 2x packed-16-bit mode.
        """
        n = cos2.shape[-1]
        sb64 = pool.tile([ROPE_DIMS, n], bf16, tag="rope_sb64")
        nc.scalar.copy(sb64, ps[0:ROPE_DIMS])
        ss = pool.tile([ROPE_DIMS, n], bf16, tag="rope_ss")
        # SWDGE (gpsimd) queue: keeps these data-dependent shuffles out of
        # the SP load queue so weight streaming is never blocked behind them
        nc.gpsimd.dma_start(out=ss[0:HALF], in_=sb64[HALF:ROPE_DIMS])
        nc.gpsimd.dma_start(out=ss[HALF:ROPE_DIMS], in_=sb64[0:HALF])
        t1 = pool.tile([ROPE_DIMS, n], bf16, tag="rope_t1")
        eng = eng or nc.vector
        eng.tensor_mul(t1, sb64, cos2)
        eng.tensor_mul(ss, ss, sin2m)
        eng.tensor_add(out_bf[0:ROPE_DIMS], t1, ss)
        nc.scalar.copy(out_bf[ROPE_DIMS:HD], ps[ROPE_DIMS:HD])

    with tile.TileContext(nc) as tc, ExitStack() as ctx:
        p_const = ctx.enter_context(tc.tile_pool(name="const", bufs=1))
        p_xt = ctx.enter_context(tc.tile_pool(name="xt", bufs=1))
        p_kv = ctx.enter_context(tc.tile_pool(name="kv", bufs=1))
        p_wq = ctx.enter_context(tc.tile_pool(name="wq", bufs=3))
        p_qt = ctx.enter_context(tc.tile_pool(name="qt", bufs=3))
        p_es = ctx.enter_context(tc.tile_pool(name="es", bufs=6))
        p_red = ctx.enter_context(tc.tile_pool(name="red", bufs=8))
        p_dn = ctx.enter_context(tc.tile_pool(name="dn", bufs=2))
        p_tmp = ctx.enter_context(tc.tile_pool(name="tmp", bufs=4))
        p_otn = ctx.enter_context(tc.tile_pool(name="otn", bufs=1))
        p_wo = ctx.enter_context(tc.tile_pool(name="wo", bufs=2))
        p_ob = ctx.enter_context(tc.tile_pool(name="ob", bufs=4))

        # ---- loads: all transfers serialize on the DMA engines, so the order
        # below IS the startup schedule; kv/q0 matmuls are paced pair-by-pair
        wk_sb = p_const.tile([128, NDT, 2, HD], f8e4, tag="wk")
        nc.sync.dma_start(out=wk_sb, in_=wk_d[:])

        # x in 8 dt-pair tiles [p, 2(dt), 2(hl), pos] so the first DoubleRow
        # starts after the first pair lands
        xp = []
        for i in range(NPAIR):
            t_ = p_xt.tile([128, 2, 2, NK], f8e4, tag=f"xp{i}")
            xp.append(t_)
        nc.sync.dma_start(out=xp[0], in_=xT_d[:, 0:2])

        wv_sb = p_const.tile([128, NDT, 2, HD], f8e4, tag="wv")
        nc.sync.dma_start(out=wv_sb, in_=wv_d[:])

        wq_sb = []
        for h in range(H):
            t_ = p_wq.tile([128, NDT, 2, 128], f8e4, tag="wq", name=f"wq{h}")
            wq_sb.append(t_)
        nc.sync.dma_start(out=wq_sb[0], in_=wq_d[0])

        cos_sb = p_const.tile([ROPE_DIMS, NK], bf16, tag="cos")
        sin_sb = p_const.tile([ROPE_DIMS, NK], bf16, tag="sin")
        msk_sb = p_const.tile([128, NQT, NSIG, 128], bf16, tag="msk")

        for i in range(1, NPAIR):
            nc.sync.dma_start(out=xp[i], in_=xT_d[:, 2 * i:2 * i + 2])
            if i == 3:
                nc.sync.dma_start(out=cos_sb, in_=cos_d[:])
                nc.sync.dma_start(out=sin_sb, in_=sin_d[:])
            elif i == 5:
                nc.sync.dma_start(out=wq_sb[1], in_=wq_d[1])
                nc.sync.dma_start(out=msk_sb, in_=msk_d[:])
        nc.sync.dma_start(out=wq_sb[2], in_=wq_d[2])

        # ---- K^T (RoPE'd) and V projections, dt-outer so PE starts on the
        # first xT tile while the rest stream in; head-0 q-projection is
        # folded in before the PSUM pool swap so PE never drains ----
        kt = p_kv.tile([128, NK], bf16, tag="kt")
        vt_sb = p_kv.tile([128, NK], bf16, tag="vt")
        v_sb = []
        for s in range(NKT):
            t_ = p_kv.tile([128, HD], bf16, tag=f"v{s}")
            v_sb.append(t_)
        qt_sb = []
        for h in range(H):
            t_ = p_qt.tile([128, CHUNK], bf16, tag="qt", name=f"qt{h}")
            qt_sb.append(t_)

        def q_main(psq, h, p_, start):
            nc.tensor.matmul(
                psq, wq_sb[h][:, 2 * p_:2 * p_ + 2, 1, :],
                xp[p_][:, :, 0, CHUNK:NK],
                start=start, stop=False, perf_mode=DR,
            )

        def q_corr(psq, h, dt, stop):
            p_, s_ = divmod(dt, 2)
            nc.tensor.matmul(
                psq, wq_sb[h][:, dt, :, :], xp[p_][:, s_, :, CHUNK:NK],
                start=False, stop=stop, perf_mode=DR,
            )

        def q_rope(psq, h):
            _rope(p_tmp, qt_sb[h], psq, cos_sb[:, CHUNK:NK], sin_sb[:, CHUNK:NK],
                  eng=nc.gpsimd)

        def q_proj(ps_qp, h):
            psq = ps_qp.tile([128, CHUNK], fp32, tag="ps_q")
            for p_ in range(NPAIR):
                q_main(psq, h, p_, p_ == 0)
            for dt in range(NDT):
                q_corr(psq, h, dt, dt == NDT - 1)
            q_rope(psq, h)

        with tc.tile_pool(name="ps_q", bufs=2, space=bass.MemorySpace.PSUM) as ps_qp:
            with tc.tile_pool(
                name="ps_kv", bufs=1, space=bass.MemorySpace.PSUM
            ) as ps_kv:
                ps_k = [
                    ps_kv.tile([128, 512], fp32, tag=f"ps_k{i}", name=f"ps_k{i}")
                    for i in range(2)
                ]
                # V is computed transposed (one PSUM accumulation group per
                # bank — concurrent groups within a bank are illegal) and
                # tile-transposed to [pos, dv] via the DMA xbar afterwards
                ps_vt = [
                    ps_kv.tile([128, 512], fp32, tag=f"ps_vt{i}", name=f"ps_vt{i}")
                    for i in range(2)
                ]
                # kv + head-0 q matmuls interleaved pair-by-pair, paced by the
                # x DMA stream (13 DoubleRows per pair ~= one chunk DMA)
                psq0 = ps_qp.tile([128, CHUNK], fp32, tag="ps_q")
                for p_ in range(NPAIR):
                    st = p_ == 0
                    for nh in range(2):
                        cols = slice(nh * 512, (nh + 1) * 512)
                        nc.tensor.matmul(
                            ps_k[nh], wk_sb[:, 2 * p_:2 * p_ + 2, 1, :],
                            xp[p_][:, :, 0, cols], start=st, stop=False,
                            perf_mode=DR,
                        )
                        nc.tensor.matmul(
                            ps_vt[nh], wv_sb[:, 2 * p_:2 * p_ + 2, 1, :],
                            xp[p_][:, :, 0, cols], start=st, stop=False,
                            perf_mode=DR,
                        )
                    q_main(psq0, 0, p_, st)
                    for s_ in range(2):
                        dt = 2 * p_ + s_
                        sp = dt == NDT - 1
                        for nh in range(2):
                            cols = slice(nh * 512, (nh + 1) * 512)
                            nc.tensor.matmul(
                                ps_k[nh], wk_sb[:, dt, :, :],
                                xp[p_][:, s_, :, cols], start=False, stop=sp,
                                perf_mode=DR,
                            )
                            nc.tensor.matmul(
                                ps_vt[nh], wv_sb[:, dt, :, :],
                                xp[p_][:, s_, :, cols], start=False, stop=sp,
                                perf_mode=DR,
                            )
                        q_corr(psq0, 0, dt, sp)
                q_rope(psq0, 0)
                for nh in range(2):
                    cols = slice(nh * 512, (nh + 1) * 512)
                    _rope(p_tmp, kt[:, cols], ps_k[nh],
                          cos_sb[:, cols], sin_sb[:, cols])
                    # scaled copy on Act: divides out the 2^15 fp8 psum factor
                    # and bakes in the SO=64 scale the otn fp8 quantization
                    # wants, so the attention-output path needs no extra ops
                    nc.scalar.activation(
                        vt_sb[:, cols], ps_vt[nh], AF.Copy, scale=SO / S2
                    )
                for c in range(NKT):
                    nc.sync.dma_start_transpose(
                        out=v_sb[c], in_=vt_sb[:, c * 128:(c + 1) * 128]
                    )
                # head-1 q-projection keeps PE busy while K-rope / V copies
                # drain the kv PSUM tiles (head-0 is folded into the kv loop)
                q_proj(ps_qp, 1)

            # ---- per-head attention (software-pipelined PE stream) ----
            # attention outputs in fp8 (hi, lo) head-pair tiles, ready to be
            # DoubleRow o-proj operands: [dv, 2(head-in-pair), 2(hi/lo), q]
            otn_p = []
            for j in range(H // 2):
                t_ = p_otn.tile([128, 2, 2, CHUNK], f8e4, tag=f"otnp{j}")
                otn_p.append(t_)

            with (
                tc.tile_pool(name="ps_s", bufs=2, space=bass.MemorySpace.PSUM) as ps_sp,
                tc.tile_pool(name="ps_o", bufs=2, space=bass.MemorySpace.PSUM) as ps_op,
            ):
                bias_sb = p_const.tile([128, D], bf16, tag="bias")
                wo_tiles = []
                for n in range(DN):
                    t_ = p_wo.tile([128, H, 2, 512], f8e4, tag="wo", name=f"wo{n}")
                    wo_tiles.append(t_)

                pso_br = []
                for h in range(H):
                    if h == 12:
                        # prefetch o-proj operands so the tail phase starts hot
                        nc.sync.dma_start(
                            out=bias_sb,
                            in_=bass.AP(tensor=bo_d, offset=0, ap=[[0, 128], [1, D]]),
                        )
                    if h == 14:
                        nc.sync.dma_start(out=wo_tiles[0], in_=wo_d[0])
                    qt = qt_sb[h]
                    otp = ps_op.tile([128, CHUNK], fp32, tag="ps_o")
                    dn = p_dn.tile([128, NQT, 128], fp32, tag="dn")
                    ess = []

                    def attn_unit(t):
                        pss = ps_sp.tile([128, NSIG, 128], fp32, tag="ps_s")
                        qsl = qt[:, t * 128:(t + 1) * 128]
                        for sig in range(NSIG):
                            s = t + sig
                            nc.tensor.matmul(
                                pss[:, sig, :], kt[:, s * 128:(s + 1) * 128], qsl,
                                start=True, stop=True,
                            )
                        es = p_es.tile([128, NSIG, 128], bf16, tag="es")
                        ess.append(es)
                        # q and k both carry the 2^15 fp8 psum factor; the exp
                        # scale divides it back out
                        nc.scalar.activation(es, pss, AF.Exp, scale=SCALE / (S2 * S2))
                        nc.vector.tensor_mul(es, es, msk_sb[:, t, :, :])
                        # fp16 add-tree: 2-byte packed DVE ops run 2x; exact
                        # zeros from the mask keep the boundary cores exact
                        r2 = p_red.tile([128, 2, 128], fp16, tag="r2")
                        nc.gpsimd.tensor_add(r2, es[:, 0:2, :], es[:, 2:4, :])
                        r1 = p_red.tile([128, 128], fp16, tag="r1")
                        nc.vector.tensor_add(r1, r2[:, 0, :], r2[:, 1, :])
                        red = p_red.tile([128, 128], fp16, tag="red")
                        nc.vector.tensor_add(red, r1, es[:, 4, :])
                        nc.gpsimd.partition_all_reduce(
                            dn[:, t, :], red, channels=128,
                            reduce_op=bass_isa.ReduceOp.add,
                        )

                    last_head = h == H - 1
                    if h == H - 2:
                        # the next-next-head q-projection no longer exists to
                        # cover the exp/mask chain; instead accumulate heads
                        # 0..14 of the first two o-proj units into the (free)
                        # ps_q slots, leaving the groups open until otn[15]
                        for i in range(2):
                            t_ = ps_qp.tile(
                                [128, CHUNK], fp32, tag="ps_q", name=f"ps_br{i}"
                            )
                            pso_br.append(t_)

                    def oproj_mains(pso, n, tq, hs, js, start):
                        for j in js:
                            nc.tensor.matmul(
                                pso, otn_p[j][:, :, 0, tq],
                                wo_tiles[n][:, 2 * j:2 * j + 2, 1, hs],
                                start=(start and j == js[0]), stop=False,
                                perf_mode=DR,
                            )

                    def oproj_corrs(pso, n, tq, hs, h2s, stop):
                        for h2 in h2s:
                            j, s_ = divmod(h2, 2)
                            nc.tensor.matmul(
                                pso, otn_p[j][:, s_, :, tq],
                                wo_tiles[n][:, h2, :, hs],
                                start=False, stop=(stop and h2 == h2s[-1]),
                                perf_mode=DR,
                            )

                    def oproj_partial(i, part):
                        # PE executes in queue order, so each part may only
                        # touch otn written by strictly earlier heads:
                        # part 0 (issued during head 14) heads 0..13,
                        # part 1 (during head 15) head 14,
                        # part 2 (after otn[15]) pair-7 main + corr 15
                        tq = slice(i * 128, (i + 1) * 128)
                        pso = pso_br[i][:, 0:512]
                        if part == 0:
                            oproj_mains(pso, 0, tq, slice(0, 512),
                                        list(range(7)), True)
                            oproj_corrs(pso, 0, tq, slice(0, 512),
                                        list(range(14)), False)
                        elif part == 1:
                            oproj_corrs(pso, 0, tq, slice(0, 512), [14], False)
                        else:
                            oproj_mains(pso, 0, tq, slice(0, 512), [7], False)
                            oproj_corrs(pso, 0, tq, slice(0, 512), [15], True)

                    if h + 3 < H:
                        nc.sync.dma_start(out=wq_sb[h + 3], in_=wq_d[h + 3])
                    for t in range(2):
                        attn_unit(t)
                    if h + 2 < H:
                        q_proj(ps_qp, h + 2)
                    elif h == H - 2:
                        oproj_partial(0, 0)
                    else:
                        oproj_partial(0, 1)
                        oproj_partial(1, 0)
                        oproj_partial(1, 1)
                    for t in range(2, NQT):
                        attn_unit(t)
                    for t in range(NQT):
                        for sig in range(NSIG):
                            nc.tensor.matmul(
                                otp[:, t * 128:(t + 1) * 128],
                                v_sb[t + sig], ess[t][:, sig, :],
                                start=(sig == 0), stop=(sig == NSIG - 1),
                            )
                    rview = dn.rearrange("p t q -> p (t q)")
                    nc.vector.reciprocal(rview, rview)
                    # otn = 64 * normalized head output (vt pre-carried SO/S2);
                    # hi/lo fp8 extraction rides the Pool engine
                    o16 = p_tmp.tile([128, CHUNK], fp16, tag="o16")
                    nc.vector.tensor_mul(o16, otp, rview)
                    j_, s_ = divmod(h, 2)
                    nc.gpsimd.tensor_copy(otn_p[j_][:, s_, 0, :], o16)
                    nc.gpsimd.tensor_sub(
                        otn_p[j_][:, s_, 1, :], o16, otn_p[j_][:, s_, 0, :]
                    )
                    if last_head:
                        for i in range(2):
                            oproj_partial(i, 2)
                            ob = p_ob.tile([128, 512], bf16, tag="ob")
                            nc.vector.tensor_add(
                                ob, pso_br[i], bias_sb[:, 0:512]
                            )
                            nc.scalar.dma_start(
                                out=out_d[i * 128:(i + 1) * 128, 0:512], in_=ob
                            )

                def oproj_slice(pso, n, t, hs):
                    tq = slice(t * 128, (t + 1) * 128)
                    for j in range(H // 2):
                        nc.tensor.matmul(
                            pso[:, hs], otn_p[j][:, :, 0, tq],
                            wo_tiles[n][:, 2 * j:2 * j + 2, 1, hs],
                            start=(j == 0), stop=False, perf_mode=DR,
                        )
                    for h in range(H):
                        j, s_ = divmod(h, 2)
                        nc.tensor.matmul(
                            pso[:, hs], otn_p[j][:, s_, :, tq],
                            wo_tiles[n][:, h, :, hs],
                            start=False, stop=(h == H - 1), perf_mode=DR,
                        )
                    ob = p_ob.tile([128, 512], bf16, tag="ob")
                    nc.vector.tensor_add(
                        ob[:, hs], pso[:, hs],
                        bias_sb[:, n * 512 + hs.start:n * 512 + hs.stop],
                    )
                    nc.scalar.dma_start(
                        out=out_d[
                            t * 128:(t + 1) * 128,
                            n * 512 + hs.start:n * 512 + hs.stop,
                        ],
                        in_=ob[:, hs],
                    )

                def oproj_unit(pso, n, t):
                    oproj_slice(pso, n, t, slice(0, 512))

                # ---- o-projection + bias, straight out of the (now idle)
                # score-PSUM banks — no pool transition barrier ----
                nc.sync.dma_start(out=wo_tiles[1], in_=wo_d[1])
                for t in range(2, NQT):
                    pst = ps_sp.tile([128, NSIG, 128], fp32, tag="ps_s")
                    pso = pst.rearrange("p s q -> p (s q)")[:, 0:512]
                    oproj_unit(pso, 0, t)
                for n in range(1, DN):
                    if n + 1 < DN:
                        nc.sync.dma_start(out=wo_tiles[n + 1], in_=wo_d[n + 1])
                    for t in range(NQT):
                        pst = ps_sp.tile([128, NSIG, 128], fp32, tag="ps_s")
                        pso = pst.rearrange("p s q -> p (s q)")[:, 0:512]
                        if n == DN - 1 and t == NQT - 1:
                            # final unit: the last slice goes in the OTHER
                            # ps_s slot so its matmuls don't wait for the
                            # first slice's bias-add read (same-tile hazard),
                            # and the trailing store chain is short
                            oproj_slice(pso, n, t, slice(0, 384))
                            pst2 = ps_sp.tile(
                                [128, NSIG, 128], fp32, tag="ps_s", name="pst2"
                            )
                            pso2 = pst2.rearrange("p s q -> p (s q)")[:, 0:512]
                            oproj_slice(pso2, n, t, slice(384, 512))
                        else:
                            oproj_unit(pso, n, t)

    nc.compile()
    return nc


def _get_program():
    global _PROGRAM
    if _PROGRAM is None:
        _PROGRAM = _build_program()
    return _PROGRAM


def _q8pair(a, s):
    """fp8 e4m3 (hi, lo) pair of a*s; lo is quantized at the same scale."""
    hi = (a * s).astype(F8)
    lo = (a * s - hi.astype(np.float32)).astype(F8)
    return hi, lo


def _make_in_maps(x, Wq, Wk, Wv, Wo, bo):
    # host pre-layouts that mirror the SBUF tiles exactly (partition-major,
    # >=512B contiguous per partition) so every DMA runs at full bus rate.
    # q/k/v weights ship as fp8 (lo, hi) pairs in [p, dt, hl, col] layout.
    def wpair(W, ncol):
        hi, lo = _q8pair(np.asarray(W, np.float32), SW)
        # [hl, dt, p, col] -> [p, dt, hl, col]
        st = np.stack([lo, hi]).reshape(2, NDT, 128, ncol)
        return np.ascontiguousarray(st.transpose(2, 1, 0, 3))

    Wq_b = np.ascontiguousarray(
        wpair(Wq, D).reshape(128, NDT, 2, H, 128).transpose(3, 0, 1, 2, 4)
    )
    Wk_b = wpair(Wk, HD)
    Wv_b = wpair(Wv, HD)
    whi, wlo = _q8pair(np.asarray(Wo, np.float32), SWO)
    # [hl, h, p, n, c] -> [n, p, h, hl, c]
    Wo_b = np.ascontiguousarray(
        np.stack([wlo, whi]).reshape(2, H, 128, DN, 512).transpose(3, 2, 1, 0, 4)
    )
    bo_f = np.ascontiguousarray(
        np.asarray(bo, np.float32).reshape(1, D) * (SO * SWO)
    ).astype(BF16)

    inv_freq = np.exp(
        -np.log(np.float32(ROPE_BASE))
        * (np.arange(0, ROPE_DIMS, 2, dtype=np.float32) / np.float32(ROPE_DIMS))
    ).astype(np.float32)

    in_maps = []
    for c in range(8):
        b, g = divmod(c, 4)
        k_start = 512 * g - 512
        xs = np.zeros((NK, D), np.float32)
        lo = max(0, k_start)
        xs[lo - k_start:] = x[b, lo:k_start + NK]
        xhi, xlo = _q8pair(xs.T, SX)
        # [hl, dt, p, pos] -> [p, dt, hl, pos]
        xT = np.ascontiguousarray(
            np.stack([xhi, xlo]).reshape(2, NDT, 128, NK).transpose(2, 1, 0, 3)
        )

        pos = (k_start + np.arange(NK)).astype(np.float32)
        theta = pos[None, :] * inv_freq[:, None]          # [32, NK]
        cos2 = np.ascontiguousarray(
            np.concatenate([np.cos(theta)] * 2, axis=0)).astype(BF16)
        sin2 = np.ascontiguousarray(
            np.concatenate([-np.sin(theta), np.sin(theta)], axis=0)).astype(BF16)

        m = np.zeros((NQT, NSIG, 128, 128), np.float32)
        for t in range(NQT):
            Tg = NQT * g + t
            for sig in range(NSIG):
                S = Tg - 4 + sig
                if S < 0:
                    continue
                i = (128 * Tg + np.arange(128))[None, :]   # queries (cols)
                j = (128 * S + np.arange(128))[:, None]    # keys (rows)
                m[t, sig] = (((i - j) >= 0) & ((i - j) < WINDOW)).astype(np.float32)
        # SBUF layout [k, t, sig, q]
        masks = np.ascontiguousarray(m.transpose(2, 0, 1, 3)).astype(BF16)

        in_maps.append({
            "xT8": xT, "Wq": Wq_b, "Wk": Wk_b, "Wv": Wv_b, "Wo": Wo_b,
            "bo": bo_f, "cosT": cos2, "sinT": sin2, "masks": masks,
        })
    return in_maps


def _unshard(results):
    out = np.zeros((B, L, D), np.float32)
    for c in range(8):
        b, g = divmod(c, 4)
        # the o-proj psum carries the otn (64x) and Wo (2048x) fp8 scales
        out[b, CHUNK * g:CHUNK * (g + 1)] = (
            results[c]["out"].astype(np.float32) / (SO * SWO)
        )
    return out


def kernel(x, Wq, Wk, Wv, Wo, bo):
    from concourse.bass_utils import run_bass_kernel_spmd

    nc = _get_program()
    in_maps = _make_in_maps(x, Wq, Wk, Wv, Wo, bo)
    res = run_bass_kernel_spmd(nc, in_maps, core_ids=list(range(8)))
    return _unshard(res.results)

